# revision 1
# baseline (speedup 1.0000x reference)
"""AttentionBlock (GroupNorm -> QKV -> full attention -> out-proj + residual)
for B=4, C=128, N=4096 on 8 Trainium2 NeuronCores.

Sharding: 8 cores = 4 batches x 2 query-slabs of N/2. Every core runs the
same program; the host rolls each core's x so its query slab is always
columns [0, N/2).

Key moves:
- q/k are never materialized: scores = h^T (w_q^T w_k) h + h^T (w_k^T b_q)
  with the weight product composed on the host, and the k-bias dropped
  (softmax is invariant to per-query constants). One slab projection
  qt = M^T h + bqt feeds all QK matmuls with h itself as the stationary side.
- Matmuls run in float32r (fp32 data, PE rounds to ~tf32 at full speed);
  exp'd probabilities are stored fp8e4m3 and the PV matmul contracts two
  128-key tiles per instruction with fp8 DoubleRow (2x PE throughput).
- Scores are computed transposed [j, i] so exp feeds PV with no transposes;
  softmax row sums come from all-ones matmuls over DVE-pairsummed P tiles
  accumulated in PSUM next to PV, normalized at the end of each pass by
  reciprocal_approx_fast + one multiply.
- PE executes in program order, so PV/rowsum work for a tile pair is emitted
  one tile late, keeping PE busy while ACT runs exp (software pipelining).
- The v projection absorbs the GroupNorm affine (vT = xB^T (a o w_v), with the
  shift folded through attention into the output bias), so the vT pipeline
  runs off raw bf16 x during the stats phase instead of waiting for h.
End-to-end relative error vs the fp32 reference is ~2e-4 (fp8-dominated);
cost-model (TimelineSim) per-core time ~94us.
"""

import math
import sys

if "/opt/trn_rl_repo" not in sys.path:
    sys.path.insert(0, "/opt/trn_rl_repo")

import numpy as np

C = 128
G = 8
GS = C // G  # channels per group
EPS = 1e-5
N_CORES = 8


def build(N=4096, repeat=1):
    """Build the per-core Bass program. Returns the compiled Bacc module."""
    import concourse.bacc as bacc
    import concourse.bass as bass
    import concourse.mybir as mybir
    import concourse.tile as tile

    f32 = mybir.dt.float32
    f32r = mybir.dt.float32r
    AF = mybir.ActivationFunctionType
    OP = mybir.AluOpType

    S = N // 2           # query slab width per core
    ICW = min(1024, S)   # i-chunk width (one PV/rowsum accumulation pass)
    NIC = S // ICW       # number of i-chunk passes
    NJT = N // 128       # number of j (key) tiles
    BNC = min(512, N)    # bn_stats chunk
    NBN = N // BNC
    PCW = min(512, S)    # projection/epilogue chunk width for slab-sized tensors
    NPC = S // PCW
    SCALE = 1.0 / math.sqrt(C)

    nc = bacc.Bacc("TRN2", target_bir_lowering=False, debug=False)

    x_d = nc.dram_tensor("x", [C, N], f32, kind="ExternalInput").ap()
    w_d = nc.dram_tensor("wcat", [C, 4 * C], f32, kind="ExternalInput").ap()
    m_d = nc.dram_tensor("gmask", [C, C], f32, kind="ExternalInput").ap()
    b_d = nc.dram_tensor("bcat", [C, 5], f32, kind="ExternalInput").ap()
    o_d = nc.dram_tensor("out", [C, S], f32, kind="ExternalOutput").ap()

    with tile.TileContext(nc) as tc:
        with tc.tile_pool(name="consts", bufs=1) as cp, \
             tc.tile_pool(name="big", bufs=1) as bp, \
             tc.tile_pool(name="small", bufs=3) as sp_, \
             tc.tile_pool(name="pP", bufs=6) as pP:
            _loop = tc.For_i(0, repeat, 1) if repeat > 1 else None
            if _loop is not None:
                _loop.__enter__()

            # ---- loads + constants ----
            bf16 = mybir.dt.bfloat16
            xS = bp.tile([C, N], f32, tag="x")
            for dc in range(NBN):
                nc.sync.dma_start(xS[:, dc * BNC:(dc + 1) * BNC],
                                  x_d[:, dc * BNC:(dc + 1) * BNC])
            xB = bp.tile([C, N], bf16, tag="xB")
            for dc in range(NBN):
                nc.gpsimd.tensor_copy(out=xB[:, dc * BNC:(dc + 1) * BNC],
                                      in_=xS[:, dc * BNC:(dc + 1) * BNC])
            wS = cp.tile([C, 4 * C], f32, tag="w")
            nc.sync.dma_start(wS[:], w_d[:])
            wR = cp.tile([C, 4 * C], f32r, tag="wr")
            nc.vector.tensor_copy(wR[:], wS[:])
            mS = cp.tile([C, C], f32, tag="gmask")
            nc.sync.dma_start(mS[:], m_d[:])
            bS = cp.tile([C, 5], f32, tag="bcat")
            nc.sync.dma_start(bS[:], b_d[:])
            onesS = cp.tile([C, C], f32, tag="ones")
            nc.vector.memset(onesS[:], 1.0)
            onesR = cp.tile([C, C], f32r, tag="onesr")
            nc.vector.tensor_copy(onesR[:], onesS[:])
            epsT = cp.tile([C, 1], f32, tag="eps")
            nc.vector.memset(epsT[:], EPS)
            f8 = mybir.dt.float8e4
            onesF8 = cp.tile([C, C], f8, tag="onesf8")
            nc.vector.tensor_copy(onesF8[:], onesS[:])

            hR = bp.tile([C, N], f32r, tag="h")
            qtR = bp.tile([C, S], f32r, tag="qt")
            vTR = bp.tile([C, N], f8, tag="vT")
            h2nR = bp.tile([C, S], f32r, tag="h2n")
            outS = bp.tile([C, S], f32, tag="outS")

            with tc.tile_pool(name="ps_pre", bufs=2, space="PSUM") as pre, \
                 tc.tile_pool(name="ps_vt", bufs=2, space="PSUM") as pvt:
                # ---- GroupNorm stats ----
                st6 = sp_.tile([C, NBN, 6], f32, tag="st6")
                for i in range(NBN):
                    nc.vector.bn_stats(out=st6[:, i, :], in_=xS[:, i * BNC:(i + 1) * BNC])
                mv = sp_.tile([C, 2], f32, tag="mv")
                nc.vector.bn_aggr(out=mv[:], in_=st6[:])
                # mv col1 <- mean^2 + var = E[x^2] (in place)
                nc.vector.scalar_tensor_tensor(out=mv[:, 1:2], in0=mv[:, 0:1],
                                               scalar=mv[:, 0:1], in1=mv[:, 1:2],
                                               op0=OP.mult, op1=OP.add)
                # cross-partition group reduce: gstats[c,:] = [gmean, gEx2] of c's group
                gps = pre.tile([C, 2], f32, tag="gstats")
                nc.tensor.matmul(gps[:], mS[:], mv[:], start=True, stop=True)
                gst = sp_.tile([C, 2], f32, tag="gst")
                nc.vector.tensor_copy(gst[:], gps[:])
                # xv = eps + gEx2 - gmean^2  (group variance + eps)
                i32 = mybir.dt.int32
                gv = sp_.tile([C, 1], f32, tag="gv")
                nc.vector.scalar_tensor_tensor(out=gv[:], in0=gst[:, 0:1],
                                               scalar=gst[:, 0:1], in1=gst[:, 1:2],
                                               op0=OP.mult, op1=OP.subtract)
                xv = sp_.tile([C, 1], f32, tag="xv")
                nc.vector.tensor_tensor(out=xv[:], in0=epsT[:], in1=gv[:], op=OP.subtract)
                magicT = cp.tile([C, 1], i32, tag="magic")
                nc.vector.memset(magicT[:], 0x5F3759DF)
                yh = sp_.tile([C, 1], i32, tag="yh")
                nc.vector.tensor_scalar(out=yh[:], in0=xv[:].bitcast(i32), scalar1=1,
                                        scalar2=None, op0=OP.logical_shift_right)
                nc.vector.tensor_tensor(out=yh[:], in0=magicT[:], in1=yh[:], op=OP.subtract)
                inv = sp_.tile([C, 1], f32, tag="inv")
                nc.vector.tensor_copy(inv[:], yh[:].bitcast(f32))
                tN = sp_.tile([C, 1], f32, tag="tN")
                for _ in range(2):
                    nc.vector.tensor_tensor(out=tN[:], in0=inv[:], in1=inv[:], op=OP.mult)
                    nc.vector.tensor_tensor(out=tN[:], in0=tN[:], in1=xv[:], op=OP.mult)
                    nc.vector.tensor_scalar(out=tN[:], in0=tN[:], scalar1=-0.5,
                                            scalar2=1.5, op0=OP.mult, op1=OP.add)
                    nc.vector.tensor_tensor(out=inv[:], in0=inv[:], in1=tN[:], op=OP.mult)
                aT = sp_.tile([C, 1], f32, tag="aT")
                nc.vector.tensor_tensor(out=aT[:], in0=bS[:, 3:4], in1=inv[:], op=OP.mult)
                bT = sp_.tile([C, 1], f32, tag="bT")
                nc.vector.tensor_tensor(out=bT[:], in0=gst[:, 0:1], in1=aT[:], op=OP.mult)
                nc.vector.tensor_tensor(out=bT[:], in0=bS[:, 4:5], in1=bT[:], op=OP.subtract)
                # v absorbs the GN affine: v = (a.w_v)^T x + w_v^T b; the
                # constant term rides through attention as a per-channel
                # offset on h2n and folds into the output bias via W_out.
                wvAB = cp.tile([C, C], bf16, tag="wvAB")
                nc.vector.tensor_scalar(out=wvAB[:], in0=wS[:, 2 * C:3 * C],
                                        scalar1=aT[:], scalar2=None, op0=OP.mult)
                vc_p = pre.tile([C, 2], f32, tag="gstats", name="vc_p")
                nc.tensor.matmul(vc_p[:, 0:1], wS[:, 2 * C:3 * C], bT[:],
                                 start=True, stop=True)
                vcS = sp_.tile([C, 1], f32, tag="vcS")
                nc.vector.tensor_copy(vcS[:], vc_p[:, 0:1])
                wov_p = pre.tile([C, 2], f32, tag="gstats", name="wov_p")
                nc.tensor.matmul(wov_p[:, 0:1], wS[:, 3 * C:4 * C], vcS[:],
                                 start=True, stop=True)
                beffT = sp_.tile([C, 1], f32, tag="beffT")
                nc.vector.tensor_tensor(out=beffT[:], in0=bS[:, 2:3],
                                        in1=wov_p[:, 0:1], op=OP.add)
                # h = a*x + b (f32r); first NPC chunks feed qt, the rest only
                # feed QK weights for late tiles so they can trail the vT copies
                for c in range(NPC):
                    nc.vector.tensor_scalar(out=hR[:, c * BNC:(c + 1) * BNC],
                                            in0=xS[:, c * BNC:(c + 1) * BNC],
                                            scalar1=aT[:], scalar2=bT[:],
                                            op0=OP.mult, op1=OP.add)

                # ---- projections ----
                for c in range(NPC):
                    # qt = (w_q^T w_k)^T h + w_k^T b_q;  scores = h^T qt
                    qtp = pre.tile([C, PCW], f32, tag="qtp")
                    nc.tensor.matmul(qtp[:], wR[:, 0:C], hR[:, PCW * c:PCW * (c + 1)],
                                     start=True, stop=True)
                    nc.scalar.activation(out=qtR[:, PCW * c:PCW * (c + 1)], in_=qtp[:],
                                         func=AF.Identity, bias=bS[:, 0:1], scale=1.0)
                VTW = min(1024, N)
                for g in range(N // VTW):
                    vtp = pvt.tile([C, VTW], f32, tag="vtp")
                    for jj in range(VTW // 128):
                        tj = (VTW // 128) * g + jj
                        nc.tensor.matmul(vtp[:, 128 * jj:128 * (jj + 1)],
                                         xB[:, 128 * tj:128 * (tj + 1)], wvAB[:],
                                         start=True, stop=True)
                    nc.vector.tensor_copy(vTR[:, VTW * g:VTW * (g + 1)], vtp[:])
                for c in range(NPC, N // BNC):
                    nc.vector.tensor_scalar(out=hR[:, c * BNC:(c + 1) * BNC],
                                            in0=xS[:, c * BNC:(c + 1) * BNC],
                                            scalar1=aT[:], scalar2=bT[:],
                                            op0=OP.mult, op1=OP.add)


            # ---- attention ----
            # sT double-buffered at STW wide (one exp op per tile). PE runs in
            # program order, so PV/ones for tile t are emitted one tile late:
            # while exp(t) runs on ACT, PE issues QK(t+1) instead of stalling.
            STW = min(1024, ICW)   # sT tile / exp chunk width
            NST = ICW // STW
            MMW = min(512, STW)    # matmul free-dim chunk
            NMM = STW // MMW
            NPAIR = NJT // 2  # rowsum matmuls run on pairwise P-sums (DVE adds)
            with tc.tile_pool(name="ps_sT", bufs=2, space="PSUM") as psT, \
                 tc.tile_pool(name="ps_rs", bufs=1, space="PSUM") as prs, \
                 tc.tile_pool(name="ps_h2", bufs=1, space="PSUM") as ph2:
                acc = {}        # ic -> (h2p, rsp)
                pend_pv = None  # (ic, odd t, Ppair) awaiting PV emission
                pend_ones = None  # (ic, pair_idx, Ps2) awaiting ones-MM emission

                def emit_pv(p):
                    # fp8 DoubleRow: one matmul contracts the pair of j-tiles
                    # (tp-1, tp); called only at odd tp.
                    icp, tp, Ppair = p
                    h2p = acc[icp][0]
                    pi = tp // 2
                    vpair = vTR[:, 256 * pi:256 * (pi + 1)].rearrange(
                        "p (two c) -> p two c", two=2)
                    for m in range(NMM):
                        nc.tensor.matmul(
                            h2p[:, m * MMW:(m + 1) * MMW], vpair,
                            Ppair[:, :, m * MMW:(m + 1) * MMW],
                            start=(pi == 0), stop=(pi == NJT // 2 - 1),
                            perf_mode=mybir.MatmulPerfMode.DoubleRow)

                def emit_ones(p, first=None, last=None, fp8=False):
                    icp, pi, Ps2p = p
                    rsp = acc[icp][1]
                    st = first if first is not None else (pi == 0)
                    sp2 = last if last is not None else False
                    lhs = onesF8[:] if fp8 else onesR[:]
                    for m in range(NMM):
                        nc.tensor.matmul(
                            rsp[:, m * MMW:(m + 1) * MMW], lhs,
                            Ps2p[:, m * MMW:(m + 1) * MMW],
                            start=st, stop=sp2)

                def finish_pass(ic):
                    h2p, rsp = acc[ic]
                    FCW = min(512, ICW)
                    for fc in range(ICW // FCW):
                        sl_i = slice(ic * ICW + fc * FCW, ic * ICW + (fc + 1) * FCW)
                        sl_f = slice(fc * FCW, (fc + 1) * FCW)
                        recipB = sp_.tile([C, FCW], f32, tag="recipB")
                        nc.vector.reciprocal_approx_fast(out=recipB[:], in_=rsp[:, sl_f])
                        nc.vector.tensor_tensor(out=h2nR[:, sl_i], in0=h2p[:, sl_f],
                                                in1=recipB[:], op=OP.mult)

                for ic in range(NIC):
                    acc[ic] = (ph2.tile([C, ICW], f32, tag="h2u", name=f"h2u{ic}"),
                               prs.tile([C, ICW], f32, tag="rs", name=f"rs{ic}"))
                    Ppair = None
                    for t in range(NJT):
                        for c2 in range(NST):
                            i0 = ic * ICW + c2 * STW
                            sT = psT.tile([C, STW], f32, tag="sT")
                            for m in range(NMM):
                                nc.tensor.matmul(
                                    sT[:, m * MMW:(m + 1) * MMW],
                                    hR[:, 128 * t:128 * (t + 1)],
                                    qtR[:, i0 + m * MMW:i0 + (m + 1) * MMW],
                                    start=True, stop=True)
                            if t % 2 == 0:
                                Ppair = pP.tile([C, 2, STW], f8, tag="P",
                                                name=f"P{ic}_{t}")
                            nc.scalar.activation(out=Ppair[:, t % 2, :], in_=sT[:],
                                                 func=AF.Exp, scale=SCALE)
                            if t % 2 == 1:
                                pend_pv = (ic, t, Ppair)
                                continue
                            if pend_pv is not None:
                                emit_pv(pend_pv)
                                tp = pend_pv[1]
                                Pp = pend_pv[2]
                                if tp == NJT - 1:
                                    # tail of the pass: direct fp8 ones-MMs so
                                    # the rowsum doesn't wait on a DVE pair-add
                                    if pend_ones is not None:
                                        emit_ones(pend_ones)
                                        pend_ones = None
                                    emit_ones((pend_pv[0], -1, Pp[:, 0, :]),
                                              first=False, last=False, fp8=True)
                                    emit_ones((pend_pv[0], -1, Pp[:, 1, :]),
                                              first=False, last=True, fp8=True)
                                    finish_pass(pend_pv[0])
                                else:
                                    # DVE pair-sum of the two P slices just used
                                    Ps2 = pP.tile([C, STW], f32r, tag="Ps2")
                                    nc.vector.tensor_tensor(
                                        out=Ps2[:], in0=Pp[:, 0, :],
                                        in1=Pp[:, 1, :], op=OP.add)
                                    if pend_ones is not None:
                                        emit_ones(pend_ones)
                                    pend_ones = (pend_pv[0], tp // 2, Ps2)
                                pend_pv = None
                emit_pv(pend_pv)
                tp = pend_pv[1]
                Pp = pend_pv[2]
                if pend_ones is not None:
                    emit_ones(pend_ones)
                    pend_ones = None
                emit_ones((pend_pv[0], -1, Pp[:, 0, :]), first=False, last=False,
                          fp8=True)
                emit_ones((pend_pv[0], -1, Pp[:, 1, :]), first=False, last=True,
                          fp8=True)
                finish_pass(pend_pv[0])

            # ---- out projection + bias + residual ----
            with tc.tile_pool(name="ps_ep", bufs=2, space="PSUM") as pep:
                for c in range(NPC):
                    pop = pep.tile([C, PCW], f32, tag="pop")
                    nc.tensor.matmul(pop[:], wR[:, 3 * C:4 * C],
                                     h2nR[:, PCW * c:PCW * (c + 1)], start=True, stop=True)
                    nc.vector.scalar_tensor_tensor(
                        out=outS[:, PCW * c:PCW * (c + 1)], in0=pop[:], scalar=beffT[:],
                        in1=xS[:, PCW * c:PCW * (c + 1)], op0=OP.add, op1=OP.add)
                    nc.sync.dma_start(o_d[:, PCW * c:PCW * (c + 1)],
                                      outS[:, PCW * c:PCW * (c + 1)])
            if _loop is not None:
                _loop.__exit__(None, None, None)

    nc.compile()
    return nc


def host_inputs(x, gn_w, gn_b, w_qkv, b_qkv, w_out, b_out):
    """Build the 8 per-core input maps from the full problem inputs."""
    x = np.asarray(x, dtype=np.float32)
    B, _, N = x.shape
    S = N // 2
    w_qkv = np.asarray(w_qkv, np.float32)
    w_out = np.asarray(w_out, np.float32)
    b_qkv = np.asarray(b_qkv, np.float32)
    b_out = np.asarray(b_out, np.float32)
    gn_w = np.asarray(gn_w, np.float32)
    gn_b = np.asarray(gn_b, np.float32)

    # scores = h^T (w_q^T w_k) h + h^T (w_k^T b_q); the k bias is
    # softmax-invariant and dropped, q/k are never materialized on device.
    M = w_qkv[0:C].T @ w_qkv[C:2 * C]
    wcat = np.concatenate(
        [M, np.zeros((C, C), np.float32), w_qkv[2 * C:3 * C].T, w_out.T],
        axis=1).astype(np.float32)   # [C, 4C]: [M, unused, w_v^T, w_out^T]
    gidx = np.arange(C) // GS
    gmask = (gidx[:, None] == gidx[None, :]).astype(np.float32) / GS
    b_eff = b_out + w_out @ b_qkv[2 * C:3 * C]
    bqt = w_qkv[C:2 * C].T @ b_qkv[0:C]
    bcat = np.stack([bqt, b_qkv[C:2 * C], b_eff, gn_w, gn_b], axis=1)
    bcat = np.ascontiguousarray(bcat, np.float32)       # [C, 5]

    in_maps = []
    for core in range(N_CORES):
        b, half = divmod(core, 2)
        xb = np.roll(x[b], -half * S, axis=1)
        in_maps.append({"x": np.ascontiguousarray(xb), "wcat": wcat,
                        "gmask": gmask, "bcat": bcat})
    return in_maps


_NC_CACHE = {}
_RUNNER_CACHE = {}


def _make_runner(nc):
    """Compile-once runner: replicates bass2jax.run_bass_via_pjrt but keeps the
    jitted sharded callable so repeat executions skip recompilation."""
    import jax
    import concourse.mybir as mybir
    from jax.sharding import Mesh, PartitionSpec
    from jax.experimental.shard_map import shard_map
    from concourse.bass2jax import (_bass_exec_p, install_neuronx_cc_hook,
                                    partition_id_tensor)

    install_neuronx_cc_hook()
    partition_name = nc.partition_id_tensor.name if nc.partition_id_tensor else None
    in_names, out_names, out_avals, zero_shapes = [], [], [], []
    for alloc in nc.m.functions[0].allocations:
        if not isinstance(alloc, mybir.MemoryLocationSet):
            continue
        name = alloc.memorylocations[0].name
        if alloc.kind == "ExternalInput":
            if name == partition_name:
                continue
            in_names.append(name)
        elif alloc.kind == "ExternalOutput":
            out_names.append(name)
            shape = tuple(alloc.tensor_shape)
            dtype = mybir.dt.np(alloc.dtype)
            out_avals.append(jax.core.ShapedArray(shape, dtype))
            zero_shapes.append((shape, dtype))
    n_params = len(in_names)
    all_names = in_names + out_names
    if partition_name is not None:
        all_names = all_names + [partition_name]
    donate = tuple(range(n_params, n_params + len(out_names)))

    def _body(*args):
        operands = list(args)
        if partition_name is not None:
            operands.append(partition_id_tensor())
        return tuple(_bass_exec_p.bind(
            *operands, out_avals=tuple(out_avals), in_names=tuple(all_names),
            out_names=tuple(out_names), lowering_input_output_aliases=(),
            sim_require_finite=True, sim_require_nnan=True, nc=nc))

    devices = jax.devices()[:N_CORES]
    mesh = Mesh(np.asarray(devices), ("core",))
    specs = (PartitionSpec("core"),)
    sharded = jax.jit(
        shard_map(_body, mesh=mesh,
                  in_specs=specs * (n_params + len(out_names)),
                  out_specs=specs * len(out_names), check_rep=False),
        donate_argnums=donate, keep_unused=True)

    def run(in_maps):
        concat_in = [np.concatenate([np.asarray(m[nm]) for m in in_maps], axis=0)
                     for nm in in_names]
        concat_zeros = [np.zeros((N_CORES * s[0], *s[1:]), d) for s, d in zero_shapes]
        out_arrs = sharded(*concat_in, *concat_zeros)
        out_arrs = [np.asarray(a) for a in out_arrs]
        return [{nm: out_arrs[i].reshape(N_CORES, *out_avals[i].shape)[c]
                 for i, nm in enumerate(out_names)} for c in range(N_CORES)]

    return run


def get_runner(N=4096):
    if N not in _RUNNER_CACHE:
        if N not in _NC_CACHE:
            _NC_CACHE[N] = build(N)
        _RUNNER_CACHE[N] = _make_runner(_NC_CACHE[N])
    return _RUNNER_CACHE[N]


def kernel(x, gn_w, gn_b, w_qkv, b_qkv, w_out, b_out):
    from concourse._compat import axon_active

    x = np.asarray(x, dtype=np.float32)
    B, _, N = x.shape
    S = N // 2
    in_maps = host_inputs(x, gn_w, gn_b, w_qkv, b_qkv, w_out, b_out)
    if axon_active():
        results = get_runner(N)(in_maps)
    else:
        from concourse.bass_utils import run_bass_kernel_spmd

        if N not in _NC_CACHE:
            _NC_CACHE[N] = build(N)
        results = run_bass_kernel_spmd(_NC_CACHE[N], in_maps,
                                       core_ids=list(range(N_CORES))).results
    out = np.empty((B, C, N), dtype=np.float32)
    for core in range(N_CORES):
        b, half = divmod(core, 2)
        out[b, :, half * S:(half + 1) * S] = results[core]["out"]
    return out



# revision 9
# speedup vs baseline: 1.0353x; 1.0353x over previous
"""AttentionBlock (GroupNorm -> QKV -> full attention -> out-proj + residual)
for B=4, C=128, N=4096 on 8 Trainium2 NeuronCores.

Sharding: 8 cores = 4 batches x 2 query-slabs of N/2. Every core runs the
same program; the host rolls each core's x so its query slab is always
columns [0, N/2).

v2 design (vs the f32r baseline):
- QK never materializes q/k: scores = h^T (w_q^T w_k) h + bias, with the
  weight product M composed on the host. Both QK operands are fp8 in a
  packed [64, 2, .] layout so the QK matmuls run in fp8 DoubleRow (0.5
  cyc/col, 2x the f32r rate). h8/qt8 are built in normal [128, .] layout
  and repacked by SBUF->SBUF DMA.
- Softmax row sums come from fp8 DoubleRow all-ones matmuls directly on the
  exp'd P pairs (no DVE pair-add pass at all).
- The exp itself is split across three engines by a per-tile type pattern:
  'A' tiles run true exp on ACT; 'P' tiles compute exp via the Schraudolph
  int32 bit trick on DVE (tensor_scalar mult+add -> i32 = f32 bits) and
  convert to fp8 on GpSimd; 'R' pairs use the 16-bit variant (i16 bits =
  bf16) and feed PV/ones as bf16 moving data (fp8 stationary x bf16 moving
  is legal on PE; fp32 may not mix).
- GroupNorm stats run on a host-shipped bf16 copy of x; the f32 x is only
  loaded (late, overlapped) for the residual. The GN affine is folded into
  the v projection and the QK weights on device (wvAB, wMA).
- finish (1/rowsum) and the out-projection epilogue of pass 0 are deferred
  into pass 1 so only the last chunk's epilogue sits on the tail.
End-to-end relative error vs the fp32 reference ~6e-4 (fp8 + exp-trick).
"""

import math
import sys
from collections import deque

if "/opt/trn_rl_repo" not in sys.path:
    sys.path.insert(0, "/opt/trn_rl_repo")

import numpy as np

C = 128
G = 8
GS = C // G  # channels per group
EPS = 1e-5
N_CORES = 8
SCALE_C = None  # set in build from C


def default_pattern(NJT=32, NIC=2):
    """Per-tile exp engine assignment, per pass of 16 pairs.
    'F' = f32r fast-path QK + ACT exp (pass-0 warmup only);
    'A' = ACT exp; 'P' = DVE int-trick + Pool fp8 convert."""
    p0 = {0: "FF"}
    for q in range(1, 16):
        p0[q] = "PA"
    p1 = {1: "AA", 15: "AA"}
    for q in (0, 2, 3, 4, 5, 6, 7, 8, 9, 10, 11, 12, 13, 14):
        p1[q] = "PA"
    out = []
    for pp in (p0, p1):
        out.append("".join(pp.get(q, "AA") for q in range(NJT // 2)))
    return "".join(out)


def build(N=4096, pattern=None, lag=4, n_junk=20,
          fin0_t=(2, 3), epi0_t=(3, 4), tailw=(256, 256, 256, 256)):
    """Build the per-core Bass program. Returns the compiled Bacc module."""
    import concourse.bacc as bacc
    import concourse.bass as bass
    import concourse.mybir as mybir
    import concourse.tile as tile

    f32 = mybir.dt.float32
    f32r = mybir.dt.float32r
    bf16 = mybir.dt.bfloat16
    f8 = mybir.dt.float8e4
    i32 = mybir.dt.int32
    AF = mybir.ActivationFunctionType
    OP = mybir.AluOpType
    DR = mybir.MatmulPerfMode.DoubleRow

    S = N // 2           # query slab width per core
    ICW = 1024           # i-chunk width (one PV/rowsum accumulation pass)
    NIC = S // ICW
    NJT = N // 128       # number of j (key) tiles
    NPAIR = NJT // 2
    MMW = 512
    NMM = ICW // MMW
    BNC = 512            # bn_stats chunk
    NBN = N // BNC
    HCW = 1024           # h8 chunk
    SCALE = 1.0 / math.sqrt(C)
    # Schraudolph exp constants: exp(x) ~ bitcast(int(x*K + B))
    CORR = 0.043677448
    K32 = SCALE * (1 << 23) / math.log(2.0)
    B32 = float((1 << 23) * (127 - CORR))

    if pattern is None:
        pattern = default_pattern(NJT, NIC)
    assert len(pattern) == NIC * NJT

    nc = bacc.Bacc("TRN2", target_bir_lowering=False, debug=False)

    xb_d = nc.dram_tensor("xb", [C, N], bf16, kind="ExternalInput").ap()
    xf_d = nc.dram_tensor("xf", [C, S], f32, kind="ExternalInput").ap()
    # wb = [gmask | M | wvo | bcat(4)]  (wvo = (w_out @ w_v).T, so the
    # out-projection is pre-composed into the v path)
    wb_d = nc.dram_tensor("wb", [C, 3 * C + 4], f32, kind="ExternalInput").ap()
    o_d = nc.dram_tensor("out", [C, S], f32, kind="ExternalOutput").ap()

    with tile.TileContext(nc) as tc:
        with tc.tile_pool(name="consts", bufs=1) as cp, \
             tc.tile_pool(name="big", bufs=1) as bp, \
             tc.tile_pool(name="small", bufs=3) as sp_, \
             tc.tile_pool(name="pP", bufs=6) as pP, \
             tc.tile_pool(name="pT", bufs=4) as pT, \
             tc.tile_pool(name="ps_sT", bufs=2, space="PSUM") as psT, \
             tc.tile_pool(name="ps_rs", bufs=1, space="PSUM") as prs, \
             tc.tile_pool(name="ps_h2", bufs=1, space="PSUM") as ph2:

            # ---- DMA loads (few, fat: HWDGE costs ~625ns per DMA) ----
            xB = bp.tile([C, N], bf16, tag="xB")
            XDC = 1024
            for dc in range(N // XDC):
                nc.sync.dma_start(xB[:, dc * XDC:(dc + 1) * XDC],
                                  xb_d[:, dc * XDC:(dc + 1) * XDC])
            wbS = cp.tile([C, 3 * C + 4], f32, tag="wb")
            nc.sync.dma_start(wbS[:], wb_d[:])
            mS = wbS[:, 0:C]
            wM = wbS[:, C:2 * C]
            wvo = wbS[:, 2 * C:3 * C]
            bS = wbS[:, 3 * C:3 * C + 4]

            # ---- constants ----
            onesF8 = cp.tile([C, C], f8, tag="onesf8")
            nc.vector.memset(onesF8[:], 1.0)
            onesF8_2 = cp.tile([C, 2, C], f8, tag="onesf82")
            nc.vector.memset(onesF8_2[:], 1.0)
            junkM = cp.tile([C, 512], f8, tag="junkM")
            nc.vector.memset(junkM[:], 1.0)
            epsT = cp.tile([C, 1], f32, tag="eps")
            nc.vector.memset(epsT[:], EPS)
            # dummy act: trigger the exp table load early on ACT
            dumT = cp.tile([C, 1], f32, tag="dum")
            nc.scalar.activation(out=dumT[:], in_=epsT[:], func=AF.Exp, scale=1.0)

            # PE p-state warm-up: junk matmuls into the bank that later holds
            # h2u0 (write-only; h2u0 write-after-write depends on them).
            warmT = ph2.tile([C, ICW], f32, tag="h2u", name="warm")
            for jk in range(n_junk):
                nc.tensor.matmul(warmT[:, (jk % 2) * MMW:(jk % 2 + 1) * MMW],
                                 onesF8[:], junkM[:], start=True, stop=True)

            # ---- big SBUF tensors ----
            h8 = bp.tile([C, N], f8, tag="h8")
            hRf = bp.tile([C, 512], f32r, tag="hRf")       # f32r fast-path keys
            qRf = bp.tile([C, ICW], f32r, tag="qRf")       # f32r fast-path qt
            ht2 = bp.tile([64, 2, N], f8, tag="ht2")       # packed QK stationary
            qt8 = bp.tile([C, S], f8, tag="qt8")
            qtt = bp.tile([64, 2, S], f8, tag="qtt")       # packed QK moving
            vTR = bp.tile([C, N], f8, tag="vT")            # [key, chan] (wo-folded)
            xfS = bp.tile([C, S], f32, tag="xf")
            outS = bp.tile([C, S], f32, tag="outS")

            wMA = cp.tile([C, C], bf16, tag="wMA")
            wvAB = cp.tile([C, C], bf16, tag="wvAB")
            aT = sp_.tile([C, 1], f32, tag="aT")
            bT = sp_.tile([C, 1], f32, tag="bT")
            dS = sp_.tile([C, 1], f32, tag="dS")
            beffT = sp_.tile([C, 1], f32, tag="beffT")

            # ================= preamble =================
            # GroupNorm stats on bf16 x
            st6 = sp_.tile([C, NBN, 6], f32, tag="st6")
            for i in range(NBN):
                nc.vector.bn_stats(out=st6[:, i, :],
                                   in_=xB[:, i * BNC:(i + 1) * BNC])
            mv = sp_.tile([C, 2], f32, tag="mv")
            nc.vector.bn_aggr(out=mv[:], in_=st6[:])
            nc.vector.scalar_tensor_tensor(out=mv[:, 1:2], in0=mv[:, 0:1],
                                           scalar=mv[:, 0:1], in1=mv[:, 1:2],
                                           op0=OP.mult, op1=OP.add)
            pre = psT.tile([C, ICW], f32, tag="sT", name="pre0")
            nc.tensor.matmul(pre[:, 0:2], mS, mv[:], start=True, stop=True)
            gst = sp_.tile([C, 2], f32, tag="gst")
            nc.vector.tensor_copy(gst[:], pre[:, 0:2])
            # xv = eps + gEx2 - gmean^2;  inv = sqrt(1/xv)
            gv = sp_.tile([C, 1], f32, tag="gv")
            nc.vector.scalar_tensor_tensor(out=gv[:], in0=gst[:, 0:1],
                                           scalar=gst[:, 0:1], in1=gst[:, 1:2],
                                           op0=OP.mult, op1=OP.subtract)
            xv = sp_.tile([C, 1], f32, tag="xv")
            nc.vector.tensor_tensor(out=xv[:], in0=epsT[:], in1=gv[:],
                                    op=OP.subtract)
            rxv = sp_.tile([C, 1], f32, tag="rxv")
            nc.vector.reciprocal(out=rxv[:], in_=xv[:])
            inv = sp_.tile([C, 1], f32, tag="inv")
            nc.scalar.activation(out=inv[:], in_=rxv[:], func=AF.Sqrt)
            nc.vector.tensor_tensor(out=aT[:], in0=bS[:, 2:3], in1=inv[:],
                                    op=OP.mult)
            nc.vector.tensor_tensor(out=bT[:], in0=gst[:, 0:1], in1=aT[:],
                                    op=OP.mult)
            nc.vector.tensor_tensor(out=bT[:], in0=bS[:, 3:4], in1=bT[:],
                                    op=OP.subtract)
            # folded weights
            nc.vector.tensor_scalar(out=wMA[:], in0=wM, scalar1=aT[:],
                                    scalar2=None, op0=OP.mult)
            nc.vector.tensor_scalar(out=wvAB[:], in0=wvo, scalar1=aT[:],
                                    scalar2=None, op0=OP.mult)
            # delta = M^T bT + bqt (qt bias); beff += wo@wv@bT = wvo^T bT
            pre2 = psT.tile([C, ICW], f32, tag="sT", name="pre1")
            nc.tensor.matmul(pre2[:, 0:1], wM, bT[:], start=True, stop=True)
            nc.vector.tensor_tensor(out=dS[:], in0=bS[:, 0:1], in1=pre2[:, 0:1],
                                    op=OP.add)
            nc.tensor.matmul(pre2[:, 4:5], wvo, bT[:], start=True, stop=True)
            nc.vector.tensor_tensor(out=beffT[:], in0=bS[:, 1:2],
                                    in1=pre2[:, 4:5], op=OP.add)

            # fast-path operands: hRf (keys 0:512 f32r), qRf (i 0:1024 f32r)
            nc.vector.tensor_scalar(out=hRf[:], in0=xB[:, 0:512],
                                    scalar1=aT[:], scalar2=bT[:],
                                    op0=OP.mult, op1=OP.add)

            def qt_mms(cc, dst, col0):
                slx = slice(cc * MMW, (cc + 1) * MMW)
                nc.tensor.matmul(dst[0:64, col0:col0 + MMW], wMA[:, 0:64],
                                 xB[:, slx], start=True, stop=True)
                nc.tensor.matmul(dst[64:128, col0:col0 + MMW], wMA[:, 64:128],
                                 xB[:, slx], start=True, stop=True)

            # qt chunks 0/1 (i-cols 0:1024): qRf on ACT, fp8 on DVE
            qtp01 = psT.tile([C, ICW], f32, tag="sT", name="qtp01")
            qt_mms(0, qtp01, 0)
            qt_mms(1, qtp01, MMW)
            nc.scalar.activation(out=qRf[:], in_=qtp01[:],
                                 func=AF.Identity, bias=dS[:], scale=1.0)
            nc.vector.tensor_scalar(out=qt8[:, 0:ICW], in0=qtp01[:],
                                    scalar1=1.0, scalar2=dS[:],
                                    op0=OP.mult, op1=OP.add)
            nc.sync.dma_start(qtt[:, 0, 0:ICW], qt8[0:64, 0:ICW])
            nc.sync.dma_start(qtt[:, 1, 0:ICW], qt8[64:128, 0:ICW])

            # h8 = fp8(aT*xB + bT); repack halves as they complete
            for hc in range(N // HCW):
                sl = slice(hc * HCW, (hc + 1) * HCW)
                nc.vector.tensor_scalar(out=h8[:, sl], in0=xB[:, sl],
                                        scalar1=aT[:], scalar2=bT[:],
                                        op0=OP.mult, op1=OP.add)
                if hc % 2 == 1:
                    sl2 = slice((hc - 1) * HCW, (hc + 1) * HCW)
                    nc.sync.dma_start(ht2[:, 0, sl2], h8[0:64, sl2])
                    nc.sync.dma_start(ht2[:, 1, sl2], h8[64:128, sl2])

            # v-tilde (= wo^T v) chunks: mms into psT slots; converts:
            # chunk 0 on ACT (early), 1-3 on DVE
            def vt_mms(g, vtp):
                for jj in range(HCW // 128):
                    tj = (HCW // 128) * g + jj
                    nc.tensor.matmul(vtp[:, 128 * jj:128 * (jj + 1)],
                                     xB[:, 128 * tj:128 * (tj + 1)], wvAB[:],
                                     start=True, stop=True)

            vtp0 = psT.tile([C, ICW], f32, tag="sT", name="vtp0")
            vt_mms(0, vtp0)
            nc.scalar.activation(out=vTR[:, 0:HCW], in_=vtp0[:],
                                 func=AF.Identity, scale=1.0)

            # residual x (f32), single fat DMA, overlapped with attention
            nc.sync.dma_start(xfS[:], xf_d[:])

            # ================= attention =================
            acc = {}
            pend = deque()

            def emit_pair(job):
                ic, p, ptile = job
                h2p, rsp = acc[ic]
                first = p == 0
                last = p == NPAIR - 1
                vpair = vTR[:, 256 * p:256 * (p + 1)].rearrange(
                    "p (two c) -> p two c", two=2)
                for m in range(NMM):
                    ms = slice(m * MMW, (m + 1) * MMW)
                    nc.tensor.matmul(h2p[:, ms], vpair, ptile[:, :, ms],
                                     start=first, stop=last, perf_mode=DR)
                    nc.tensor.matmul(rsp[:, ms], onesF8_2[:], ptile[:, :, ms],
                                     start=first, stop=last, perf_mode=DR)

            def fin_chunk(ic, c0, cw):
                # out[:, i] = h2u[:, i]/rs[i] + beff + x  (recip; mult; Pool stt)
                h2p, rsp = acc[ic]
                sl_i = slice(ic * ICW + c0, ic * ICW + c0 + cw)
                sl_f = slice(c0, c0 + cw)
                recipB = sp_.tile([C, cw], f32, tag=f"recipB{cw}")
                nc.vector.reciprocal_approx_fast(out=recipB[:], in_=rsp[:, sl_f])
                nc.vector.tensor_tensor(out=outS[:, sl_i], in0=h2p[:, sl_f],
                                        in1=recipB[:], op=OP.mult)

            def epi_chunk(ic, c0, cw, dma):
                sl_i = slice(ic * ICW + c0, ic * ICW + c0 + cw)
                nc.vector.scalar_tensor_tensor(
                    out=outS[:, sl_i], in0=outS[:, sl_i], scalar=beffT[:],
                    in1=xfS[:, sl_i], op0=OP.add, op1=OP.add)
                if dma:
                    d0, dw = dma
                    sl_d = slice(ic * ICW + d0, ic * ICW + d0 + dw)
                    nc.sync.dma_start(o_d[:, sl_d], outS[:, sl_d])

            # in-pass service schedule: (pass, tile) -> list of thunks
            def srv_vt(g, eng):
                def thunk():
                    spt = psT.tile([C, ICW], f32, tag="sT", name=f"vtp{g}")
                    vt_mms(g, spt)
                    if eng == "act":
                        nc.scalar.activation(out=vTR[:, HCW * g:HCW * (g + 1)],
                                             in_=spt[:], func=AF.Identity,
                                             scale=1.0)
                    else:
                        nc.vector.tensor_copy(vTR[:, HCW * g:HCW * (g + 1)],
                                              spt[:])
                return thunk

            def srv_qt_hi():
                def thunk():
                    spt = psT.tile([C, ICW], f32, tag="sT", name="qtp23")
                    qt_mms(2, spt, 0)
                    qt_mms(3, spt, MMW)
                    nc.vector.tensor_scalar(out=qt8[:, ICW:S], in0=spt[:],
                                            scalar1=1.0, scalar2=dS[:],
                                            op0=OP.mult, op1=OP.add)
                    nc.sync.dma_start(qtt[:, 0, ICW:S], qt8[0:64, ICW:S])
                    nc.sync.dma_start(qtt[:, 1, ICW:S], qt8[64:128, ICW:S])
                return thunk

            srv = {(0, 3): [srv_vt(1, "dve")], (0, 9): [srv_vt(2, "act")],
                   (0, 15): [srv_vt(3, "act")], (0, 21): [srv_qt_hi()]}
            for i, t in enumerate(fin0_t):
                srv.setdefault((1, t), []).append(
                    lambda fc=i: fin_chunk(0, fc * MMW, MMW))
            for i, t in enumerate(epi0_t):
                srv.setdefault((1, t), []).append(
                    lambda fc=i: epi_chunk(0, fc * MMW, MMW,
                                           (0, ICW) if fc == 1 else None))

            for ic in range(NIC):
                acc[ic] = (ph2.tile([C, ICW], f32, tag="h2u", name=f"h2u{ic}"),
                           prs.tile([C, ICW], f32, tag="rs", name=f"rs{ic}"))
                Ppair = None
                for t in range(NJT):
                    ty = pattern[ic * NJT + t]
                    for thunk in srv.get((ic, t), ()):
                        thunk()
                    # QK
                    sT = psT.tile([C, ICW], f32, tag="sT")
                    if ty == "F":
                        for m in range(NMM):
                            ms = slice(m * MMW, (m + 1) * MMW)
                            nc.tensor.matmul(
                                sT[:, ms], hRf[:, 128 * t:128 * (t + 1)],
                                qRf[:, ms], start=True, stop=True)
                    else:
                        hview = ht2[:, :, 128 * t:128 * (t + 1)]
                        for m in range(NMM):
                            nc.tensor.matmul(
                                sT[:, m * MMW:(m + 1) * MMW], hview,
                                qtt[:, :, ic * ICW + m * MMW:
                                    ic * ICW + (m + 1) * MMW],
                                start=True, stop=True, perf_mode=DR)
                    # exp by type
                    if t % 2 == 0:
                        Ppair = pP.tile([C, 2, ICW], f8, tag="P",
                                        name=f"P{ic}_{t}")
                    if ty in "AF":
                        nc.scalar.activation(out=Ppair[:, t % 2, :], in_=sT[:],
                                             func=AF.Exp, scale=SCALE)
                    else:  # P
                        ptmp = pT.tile([C, ICW], i32, tag="Ptmp")
                        nc.vector.tensor_scalar(out=ptmp[:], in0=sT[:],
                                                scalar1=K32, scalar2=B32,
                                                op0=OP.mult, op1=OP.add)
                        nc.gpsimd.tensor_copy(out=Ppair[:, t % 2, :],
                                              in_=ptmp[:].bitcast(f32))
                    if t % 2 == 1:
                        pend.append((ic, t // 2, Ppair))
                        while len(pend) > lag:
                            emit_pair(pend.popleft())
                while pend:
                    emit_pair(pend.popleft())

            # tail: last pass finish + epilogue in shrinking chunks
            c0 = 0
            for i, cw in enumerate(tailw):
                fin_chunk(NIC - 1, c0, cw)
                epi_chunk(NIC - 1, c0, cw, (c0, cw))
                c0 += cw
            assert c0 == ICW

    nc.compile()
    return nc


def host_inputs(x, gn_w, gn_b, w_qkv, b_qkv, w_out, b_out):
    """Build the 8 per-core input maps from the full problem inputs."""
    import ml_dtypes

    x = np.asarray(x, dtype=np.float32)
    B, _, N = x.shape
    S = N // 2
    w_qkv = np.asarray(w_qkv, np.float32)
    w_out = np.asarray(w_out, np.float32)
    b_qkv = np.asarray(b_qkv, np.float32)
    b_out = np.asarray(b_out, np.float32)
    gn_w = np.asarray(gn_w, np.float32)
    gn_b = np.asarray(gn_b, np.float32)

    # scores = h^T M h + h^T (M^T b + w_k^T b_q); q/k never materialized.
    # wvo composes the out-projection into the v path: v-tilde = wo^T v.
    M = w_qkv[0:C].T @ w_qkv[C:2 * C]
    wvo = (w_out @ w_qkv[2 * C:3 * C]).T
    gidx = np.arange(C) // GS
    gmask = (gidx[:, None] == gidx[None, :]).astype(np.float32) / GS
    bqt = w_qkv[C:2 * C].T @ b_qkv[0:C]
    b_eff = b_out + w_out @ b_qkv[2 * C:3 * C]
    bcat = np.stack([bqt, b_eff, gn_w, gn_b], axis=1)
    wb = np.concatenate([gmask, M, wvo, bcat], axis=1)
    wb = np.ascontiguousarray(wb, np.float32)           # [C, 3C+4]

    in_maps = []
    for core in range(N_CORES):
        b, half = divmod(core, 2)
        xb = np.roll(x[b], -half * S, axis=1)
        in_maps.append({
            "xb": np.ascontiguousarray(xb.astype(ml_dtypes.bfloat16)),
            "xf": np.ascontiguousarray(xb[:, :S]),
            "wb": wb})
    return in_maps


_NC_CACHE = {}
_RUNNER_CACHE = {}


def _make_runner(nc):
    """Compile-once runner: replicates bass2jax.run_bass_via_pjrt but keeps the
    jitted sharded callable so repeat executions skip recompilation."""
    import jax
    import concourse.mybir as mybir
    from jax.sharding import Mesh, PartitionSpec
    from jax.experimental.shard_map import shard_map
    from concourse.bass2jax import (_bass_exec_p, install_neuronx_cc_hook,
                                    partition_id_tensor)

    install_neuronx_cc_hook()
    partition_name = nc.partition_id_tensor.name if nc.partition_id_tensor else None
    in_names, out_names, out_avals, zero_shapes = [], [], [], []
    for alloc in nc.m.functions[0].allocations:
        if not isinstance(alloc, mybir.MemoryLocationSet):
            continue
        name = alloc.memorylocations[0].name
        if alloc.kind == "ExternalInput":
            if name == partition_name:
                continue
            in_names.append(name)
        elif alloc.kind == "ExternalOutput":
            out_names.append(name)
            shape = tuple(alloc.tensor_shape)
            dtype = mybir.dt.np(alloc.dtype)
            out_avals.append(jax.core.ShapedArray(shape, dtype))
            zero_shapes.append((shape, dtype))
    n_params = len(in_names)
    all_names = in_names + out_names
    if partition_name is not None:
        all_names = all_names + [partition_name]
    donate = tuple(range(n_params, n_params + len(out_names)))

    def _body(*args):
        operands = list(args)
        if partition_name is not None:
            operands.append(partition_id_tensor())
        return tuple(_bass_exec_p.bind(
            *operands, out_avals=tuple(out_avals), in_names=tuple(all_names),
            out_names=tuple(out_names), lowering_input_output_aliases=(),
            sim_require_finite=True, sim_require_nnan=True, nc=nc))

    devices = jax.devices()[:N_CORES]
    mesh = Mesh(np.asarray(devices), ("core",))
    specs = (PartitionSpec("core"),)
    sharded = jax.jit(
        shard_map(_body, mesh=mesh,
                  in_specs=specs * (n_params + len(out_names)),
                  out_specs=specs * len(out_names), check_rep=False),
        donate_argnums=donate, keep_unused=True)

    def run(in_maps):
        concat_in = [np.concatenate([np.asarray(m[nm]) for m in in_maps], axis=0)
                     for nm in in_names]
        concat_zeros = [np.zeros((N_CORES * s[0], *s[1:]), d) for s, d in zero_shapes]
        out_arrs = sharded(*concat_in, *concat_zeros)
        out_arrs = [np.asarray(a) for a in out_arrs]
        return [{nm: out_arrs[i].reshape(N_CORES, *out_avals[i].shape)[c]
                 for i, nm in enumerate(out_names)} for c in range(N_CORES)]

    return run


def get_runner(N=4096):
    if N not in _RUNNER_CACHE:
        if N not in _NC_CACHE:
            _NC_CACHE[N] = build(N)
        _RUNNER_CACHE[N] = _make_runner(_NC_CACHE[N])
    return _RUNNER_CACHE[N]


def kernel(x, gn_w, gn_b, w_qkv, b_qkv, w_out, b_out):
    from concourse._compat import axon_active

    x = np.asarray(x, dtype=np.float32)
    B, _, N = x.shape
    S = N // 2
    in_maps = host_inputs(x, gn_w, gn_b, w_qkv, b_qkv, w_out, b_out)
    if axon_active():
        results = get_runner(N)(in_maps)
    else:
        from concourse.bass_utils import run_bass_kernel_spmd

        if N not in _NC_CACHE:
            _NC_CACHE[N] = build(N)
        results = run_bass_kernel_spmd(_NC_CACHE[N], in_maps,
                                       core_ids=list(range(N_CORES))).results
    out = np.empty((B, C, N), dtype=np.float32)
    for core in range(N_CORES):
        b, half = divmod(core, 2)
        out[b, :, half * S:(half + 1) * S] = results[core]["out"]
    return out


# revision 19
# speedup vs baseline: 1.1652x; 1.1254x over previous
"""AttentionBlock (GroupNorm -> QKV -> full attention -> out-proj + residual)
for B=4, C=128, N=4096 on 8 Trainium2 NeuronCores.

Sharding: 8 cores = 4 batches x 2 query-slabs of N/2. Every core runs the
same program; the host rolls each core's x so its query slab is always
columns [0, N/2).

v2 design (vs the f32r baseline):
- QK never materializes q/k: scores = h^T (w_q^T w_k) h + bias, with the
  weight product M composed on the host. Both QK operands are fp8 in a
  packed [64, 2, .] layout so the QK matmuls run in fp8 DoubleRow (0.5
  cyc/col, 2x the f32r rate). h8/qt8 are built in normal [128, .] layout
  and repacked by SBUF->SBUF DMA.
- Softmax row sums come from fp8 DoubleRow all-ones matmuls directly on the
  exp'd P pairs (no DVE pair-add pass at all).
- The exp itself is split across three engines by a per-tile type pattern:
  'A' tiles run true exp on ACT; 'P' tiles compute exp via the Schraudolph
  int32 bit trick on DVE (tensor_scalar mult+add -> i32 = f32 bits) and
  convert to fp8 on GpSimd; 'R' pairs use the 16-bit variant (i16 bits =
  bf16) and feed PV/ones as bf16 moving data (fp8 stationary x bf16 moving
  is legal on PE; fp32 may not mix).
- GroupNorm stats run on a host-shipped bf16 copy of x; the f32 x is only
  loaded (late, overlapped) for the residual. The GN affine is folded into
  the v projection and the QK weights on device (wvAB, wMA).
- finish (1/rowsum) and the out-projection epilogue of pass 0 are deferred
  into pass 1 so only the last chunk's epilogue sits on the tail.
End-to-end relative error vs the fp32 reference ~6e-4 (fp8 + exp-trick).
"""

import math
import sys
from collections import deque

if "/opt/trn_rl_repo" not in sys.path:
    sys.path.insert(0, "/opt/trn_rl_repo")

import numpy as np

C = 128
G = 8
GS = C // G  # channels per group
EPS = 1e-5
N_CORES = 8
SCALE_C = None  # set in build from C


def default_pattern(NP=16, NPASS=4):
    """Per pair-tile (2 j-tiles x 512 queries) exp engine assignment, one
    char per pair-tile per pass. 'F' = f32r fast-path QK + ACT exp (first
    pass warmup); 'A' = ACT exp; 'P' = DVE int-trick + Pool fp8 convert.
    Totals: A 38 (incl 2 F), P 26."""
    p = []
    #        0123456789012345
    p.append("FFFPAPAPAPAPAPAA")  # pass 0: 3F+7A, 6P
    p.append("APAPAPAPAPAPAAPA")  # pass 1: 9A, 7P
    p.append("APAPAPAPAPAPAAPA")  # pass 2: 9A, 7P
    p.append("APAPAPAPAPAAPAAA")  # pass 3: 11A, 5P
    return "".join(p)


def build(N=4096, pattern=None, lag=4, n_junk=0,
          fin_prev_t=(2, 4), tailw=(384, 128)):
    """Build the per-core Bass program. Returns the compiled Bacc module."""
    import concourse.bacc as bacc
    import concourse.bass as bass
    import concourse.mybir as mybir
    import concourse.tile as tile

    f32 = mybir.dt.float32
    f32r = mybir.dt.float32r
    bf16 = mybir.dt.bfloat16
    f8 = mybir.dt.float8e4
    i32 = mybir.dt.int32
    AF = mybir.ActivationFunctionType
    OP = mybir.AluOpType
    DR = mybir.MatmulPerfMode.DoubleRow

    S = N // 2           # query slab width per core
    ICW = 512            # i-chunk width per pass (h2u/rs = 1 PSUM bank each)
    NPASS = S // ICW     # 4 passes over i
    NJT = N // 128
    NP = NJT // 2        # pair-tiles per pass (one [C,1024] slot each)
    BNC = 512            # bn_stats chunk
    NBN = N // BNC
    HCW = 1024           # h8 chunk
    SCALE = 1.0 / math.sqrt(C)
    # Schraudolph exp constants: exp(x) ~ bitcast(int(x*K + B))
    CORR = 0.043677448
    K32 = SCALE * (1 << 23) / math.log(2.0)
    B32 = float((1 << 23) * (127 - CORR))

    if pattern is None:
        pattern = default_pattern(NP, NPASS)
    assert len(pattern) == NPASS * NP

    nc = bacc.Bacc("TRN2", target_bir_lowering=False, debug=False)

    xb_d = nc.dram_tensor("xb", [C, N], bf16, kind="ExternalInput").ap()
    xf_d = nc.dram_tensor("xf", [C, S], f32, kind="ExternalInput").ap()
    # wb = [gmask | M | wvo | bcat(4)]  (wvo = (w_out @ w_v).T, so the
    # out-projection is pre-composed into the v path)
    wb_d = nc.dram_tensor("wb", [C, 3 * C + 4], f32, kind="ExternalInput").ap()
    o_d = nc.dram_tensor("out", [C, S], f32, kind="ExternalOutput").ap()

    with tile.TileContext(nc) as tc:
        with tc.tile_pool(name="consts", bufs=1) as cp, \
             tc.tile_pool(name="big", bufs=1) as bp, \
             tc.tile_pool(name="small", bufs=3) as sp_, \
             tc.tile_pool(name="pP", bufs=9) as pP, \
             tc.tile_pool(name="pT", bufs=5) as pT, \
             tc.tile_pool(name="ps_sT", bufs=3, space="PSUM") as psT, \
             tc.tile_pool(name="ps_rs", bufs=1, space="PSUM") as prs, \
             tc.tile_pool(name="ps_h2", bufs=1, space="PSUM") as ph2:

            # ---- DMA loads (few, fat: HWDGE costs ~625ns per DMA) ----
            xB = bp.tile([C, N], bf16, tag="xB")
            XDC = 1024
            for dc in range(N // XDC):
                nc.sync.dma_start(xB[:, dc * XDC:(dc + 1) * XDC],
                                  xb_d[:, dc * XDC:(dc + 1) * XDC])
            wbS = cp.tile([C, 3 * C + 4], f32, tag="wb")
            nc.sync.dma_start(wbS[:], wb_d[:])
            mS = wbS[:, 0:C]
            wM = wbS[:, C:2 * C]
            wvo = wbS[:, 2 * C:3 * C]
            bS = wbS[:, 3 * C:3 * C + 4]

            # ---- constants ----
            onesF8_2 = cp.tile([C, 2, C], f8, tag="onesf82")
            nc.vector.memset(onesF8_2[:], 1.0)
            epsT = cp.tile([C, 1], f32, tag="eps")
            nc.vector.memset(epsT[:], EPS)
            # dummy act: trigger the exp table load early on ACT
            dumT = cp.tile([C, 1], f32, tag="dum")
            nc.scalar.activation(out=dumT[:], in_=epsT[:], func=AF.Exp, scale=1.0)

            # ---- big SBUF tensors ----
            h8 = bp.tile([C, N], f8, tag="h8")
            hRf = bp.tile([C, 768], f32r, tag="hRf")       # f32r fast-path keys
            qRf = bp.tile([C, ICW], f32r, tag="qRf")       # f32r fast-path qt
            ht2 = bp.tile([64, 2, N], f8, tag="ht2")       # packed QK stationary
            qt8 = bp.tile([C, S], f8, tag="qt8")
            qtt = bp.tile([64, 2, S], f8, tag="qtt")       # packed QK moving
            vTR = bp.tile([C, N], f8, tag="vT")            # [key, chan] (wo-folded)
            xfS = bp.tile([C, S], f32, tag="xf")
            outS = bp.tile([C, S], f32, tag="outS")

            wMA = cp.tile([C, C], bf16, tag="wMA")
            wvAB = cp.tile([C, C], bf16, tag="wvAB")
            aT = sp_.tile([C, 1], f32, tag="aT")
            bT = sp_.tile([C, 1], f32, tag="bT")
            dS = sp_.tile([C, 1], f32, tag="dS")
            beffT = sp_.tile([C, 1], f32, tag="beffT")

            # ================= preamble =================
            # GroupNorm stats on bf16 x
            st6 = sp_.tile([C, NBN, 6], f32, tag="st6")
            for i in range(NBN):
                nc.vector.bn_stats(out=st6[:, i, :],
                                   in_=xB[:, i * BNC:(i + 1) * BNC])
            mv = sp_.tile([C, 2], f32, tag="mv")
            nc.vector.bn_aggr(out=mv[:], in_=st6[:])
            nc.vector.scalar_tensor_tensor(out=mv[:, 1:2], in0=mv[:, 0:1],
                                           scalar=mv[:, 0:1], in1=mv[:, 1:2],
                                           op0=OP.mult, op1=OP.add)
            pre = psT.tile([C, 2 * ICW], f32, tag="sT", name="pre0")
            nc.tensor.matmul(pre[:, 0:2], mS, mv[:], start=True, stop=True)
            gst = sp_.tile([C, 2], f32, tag="gst")
            nc.vector.tensor_copy(gst[:], pre[:, 0:2])
            # xv = eps + gEx2 - gmean^2;  inv = sqrt(1/xv)
            gv = sp_.tile([C, 1], f32, tag="gv")
            nc.vector.scalar_tensor_tensor(out=gv[:], in0=gst[:, 0:1],
                                           scalar=gst[:, 0:1], in1=gst[:, 1:2],
                                           op0=OP.mult, op1=OP.subtract)
            xv = sp_.tile([C, 1], f32, tag="xv")
            nc.vector.tensor_tensor(out=xv[:], in0=epsT[:], in1=gv[:],
                                    op=OP.subtract)
            rxv = sp_.tile([C, 1], f32, tag="rxv")
            nc.vector.reciprocal(out=rxv[:], in_=xv[:])
            inv = sp_.tile([C, 1], f32, tag="inv")
            nc.scalar.activation(out=inv[:], in_=rxv[:], func=AF.Sqrt)
            nc.vector.tensor_tensor(out=aT[:], in0=bS[:, 2:3], in1=inv[:],
                                    op=OP.mult)
            nc.vector.tensor_tensor(out=bT[:], in0=gst[:, 0:1], in1=aT[:],
                                    op=OP.mult)
            nc.vector.tensor_tensor(out=bT[:], in0=bS[:, 3:4], in1=bT[:],
                                    op=OP.subtract)
            # folded weights
            nc.vector.tensor_scalar(out=wMA[:], in0=wM, scalar1=aT[:],
                                    scalar2=None, op0=OP.mult)
            nc.vector.tensor_scalar(out=wvAB[:], in0=wvo, scalar1=aT[:],
                                    scalar2=None, op0=OP.mult)
            # delta = M^T bT + bqt (qt bias); beff += wo@wv@bT = wvo^T bT
            pre2 = psT.tile([C, 2 * ICW], f32, tag="sT", name="pre1")
            nc.tensor.matmul(pre2[:, 0:1], wM, bT[:], start=True, stop=True)
            nc.vector.tensor_tensor(out=dS[:], in0=bS[:, 0:1], in1=pre2[:, 0:1],
                                    op=OP.add)
            nc.tensor.matmul(pre2[:, 4:5], wvo, bT[:], start=True, stop=True)
            nc.vector.tensor_tensor(out=beffT[:], in0=bS[:, 1:2],
                                    in1=pre2[:, 4:5], op=OP.add)

            # fast-path operands: hRf (keys 0:512 f32r), qRf (i 0:512 f32r)
            nc.vector.tensor_scalar(out=hRf[:], in0=xB[:, 0:768],
                                    scalar1=aT[:], scalar2=bT[:],
                                    op0=OP.mult, op1=OP.add)

            def qt_mms(cc, dst, col0):
                slx = slice(cc * ICW, (cc + 1) * ICW)
                nc.tensor.matmul(dst[0:64, col0:col0 + ICW], wMA[:, 0:64],
                                 xB[:, slx], start=True, stop=True)
                nc.tensor.matmul(dst[64:128, col0:col0 + ICW], wMA[:, 64:128],
                                 xB[:, slx], start=True, stop=True)

            # qt chunks 0/1 (i-cols 0:1024): qRf (ACT, 512) + fp8 (DVE, 1024)
            qtp01 = psT.tile([C, 2 * ICW], f32, tag="sT", name="qtp01")
            qt_mms(0, qtp01, 0)
            qt_mms(1, qtp01, ICW)
            nc.scalar.activation(out=qRf[:], in_=qtp01[:, 0:ICW],
                                 func=AF.Identity, bias=dS[:], scale=1.0)
            nc.vector.tensor_scalar(out=qt8[:, 0:2 * ICW], in0=qtp01[:],
                                    scalar1=1.0, scalar2=dS[:],
                                    op0=OP.mult, op1=OP.add)
            nc.sync.dma_start(qtt[:, 0, 0:2 * ICW], qt8[0:64, 0:2 * ICW])
            nc.sync.dma_start(qtt[:, 1, 0:2 * ICW], qt8[64:128, 0:2 * ICW])

            # h8 = fp8(aT*xB + bT); repack halves as they complete
            for hc in range(N // HCW):
                sl = slice(hc * HCW, (hc + 1) * HCW)
                nc.vector.tensor_scalar(out=h8[:, sl], in0=xB[:, sl],
                                        scalar1=aT[:], scalar2=bT[:],
                                        op0=OP.mult, op1=OP.add)
                if hc % 2 == 1:
                    sl2 = slice((hc - 1) * HCW, (hc + 1) * HCW)
                    nc.sync.dma_start(ht2[:, 0, sl2], h8[0:64, sl2])
                    nc.sync.dma_start(ht2[:, 1, sl2], h8[64:128, sl2])

            # v-tilde (= wo^T v) chunks: mms into psT slots; chunk 0 on ACT
            def vt_mms(g, vtp):
                for jj in range(HCW // 128):
                    tj = (HCW // 128) * g + jj
                    nc.tensor.matmul(vtp[:, 128 * jj:128 * (jj + 1)],
                                     xB[:, 128 * tj:128 * (tj + 1)], wvAB[:],
                                     start=True, stop=True)

            vtp0 = psT.tile([C, 2 * ICW], f32, tag="sT", name="vtp0")
            vt_mms(0, vtp0)
            nc.scalar.activation(out=vTR[:, 0:HCW], in_=vtp0[:],
                                 func=AF.Identity, scale=1.0)

            # residual x (f32), single fat DMA, overlapped with attention
            nc.sync.dma_start(xfS[:], xf_d[:])
            xpb = bp.tile([C, S], f32, tag="xpb")
            for pc in range(2):
                nc.vector.tensor_scalar(out=xpb[:, pc * S // 2:(pc + 1) * S // 2],
                                        in0=xfS[:, pc * S // 2:(pc + 1) * S // 2],
                                        scalar1=1.0, scalar2=beffT[:],
                                        op0=OP.mult, op1=OP.add)

            # ================= attention =================
            # Each slot holds scores for a PAIR of j-tiles x ICW queries:
            # slot[:, 0:512] = j-tile 2p, slot[:, 512:1024] = j-tile 2p+1.
            # One 1024-wide exp/op1 consumes the slot; PV/rowsum are single
            # 512-wide fp8 DoubleRow matmuls into 1-bank accumulators.
            acc = {}
            pend = deque()

            def emit_pair(job):
                ps, p, ptile = job
                h2p, rsp = acc[ps]
                vpair = vTR[:, 256 * p:256 * (p + 1)].rearrange(
                    "p (two c) -> p two c", two=2)
                nc.tensor.matmul(h2p[:], vpair, ptile[:],
                                 start=p == 0, stop=p == NP - 1, perf_mode=DR)
                nc.tensor.matmul(rsp[:], onesF8_2[:], ptile[:],
                                 start=p == 0, stop=p == NP - 1, perf_mode=DR)

            def fin_chunk(ps, c0, cw):
                # out[:, i] = h2u[:, i]/rs[i] + beff + x
                h2p, rsp = acc[ps]
                sl_i = slice(ps * ICW + c0, ps * ICW + c0 + cw)
                sl_f = slice(c0, c0 + cw)
                recipB = sp_.tile([C, cw], f32, tag=f"recipB{cw}")
                nc.vector.reciprocal_approx_fast(out=recipB[:], in_=rsp[:, sl_f])
                nc.vector.tensor_tensor(out=outS[:, sl_i], in0=h2p[:, sl_f],
                                        in1=recipB[:], op=OP.mult)

            def epi_chunk(ps, c0, cw, dma):
                sl_i = slice(ps * ICW + c0, ps * ICW + c0 + cw)
                nc.gpsimd.tensor_tensor(out=outS[:, sl_i], in0=outS[:, sl_i],
                                        in1=xpb[:, sl_i], op=OP.add)
                if dma:
                    d0, dw = dma
                    sl_d = slice(ps * ICW + d0, ps * ICW + d0 + dw)
                    nc.sync.dma_start(o_d[:, sl_d], outS[:, sl_d])

            # in-pass service schedule: (pass, pair) -> list of thunks
            def srv_vt(g, eng):
                def thunk():
                    spt = psT.tile([C, 2 * ICW], f32, tag="sT", name=f"vtp{g}")
                    vt_mms(g, spt)
                    if eng == "act":
                        nc.scalar.activation(out=vTR[:, HCW * g:HCW * (g + 1)],
                                             in_=spt[:], func=AF.Identity,
                                             scale=1.0)
                    else:
                        nc.vector.tensor_copy(vTR[:, HCW * g:HCW * (g + 1)],
                                              spt[:])
                return thunk

            def srv_qt_hi():
                def thunk():
                    spt = psT.tile([C, 2 * ICW], f32, tag="sT", name="qtp23")
                    qt_mms(2, spt, 0)
                    qt_mms(3, spt, ICW)
                    nc.scalar.activation(out=qt8[:, 2 * ICW:S], in_=spt[:],
                                         func=AF.Identity, bias=dS[:],
                                         scale=1.0)
                    nc.sync.dma_start(qtt[:, 0, 2 * ICW:S], qt8[0:64, 2 * ICW:S])
                    nc.sync.dma_start(qtt[:, 1, 2 * ICW:S],
                                      qt8[64:128, 2 * ICW:S])
                return thunk

            srv = {(0, 4): [srv_vt(1, "act")], (0, 7): [srv_vt(2, "act")],
                   (0, 10): [srv_vt(3, "act")], (0, 13): [srv_qt_hi()]}
            for ps in range(1, NPASS):
                for i, q in enumerate(fin_prev_t):
                    srv.setdefault((ps, q), []).append(
                        lambda ps=ps, i=i: (fin_chunk(ps - 1, i * 256, 256),
                                            epi_chunk(ps - 1, i * 256, 256,
                                                      (0, ICW) if i == 1
                                                      else None)))

            for ps in range(NPASS):
                acc[ps] = (ph2.tile([C, ICW], f32, tag="h2u", name=f"h2u{ps}"),
                           prs.tile([C, ICW], f32, tag="rs", name=f"rs{ps}"))
                ic0 = ps * ICW
                for p in range(NP):
                    ty = pattern[ps * NP + p]
                    for thunk in srv.get((ps, p), ()):
                        thunk()
                    # QK for j-tiles 2p, 2p+1 into one [C, 1024] slot
                    sT = psT.tile([C, 2 * ICW], f32, tag="sT")
                    for k in range(2):
                        t = 2 * p + k
                        ks = slice(k * ICW, (k + 1) * ICW)
                        if ty == "F":
                            nc.tensor.matmul(
                                sT[:, ks], hRf[:, 128 * t:128 * (t + 1)],
                                qRf[:], start=True, stop=True)
                        else:
                            nc.tensor.matmul(
                                sT[:, ks], ht2[:, :, 128 * t:128 * (t + 1)],
                                qtt[:, :, ic0:ic0 + ICW],
                                start=True, stop=True, perf_mode=DR)
                    # exp: one 1024-wide op -> Ppair [C, 2, 512] fp8
                    Ppair = pP.tile([C, 2, ICW], f8, tag="P", name=f"P{ps}_{p}")
                    if ty in "AF":
                        nc.scalar.activation(out=Ppair[:], in_=sT[:],
                                             func=AF.Exp, scale=SCALE)
                    else:  # P
                        ptmp = pT.tile([C, 2 * ICW], i32, tag="Ptmp")
                        nc.vector.tensor_scalar(out=ptmp[:], in0=sT[:],
                                                scalar1=K32, scalar2=B32,
                                                op0=OP.mult, op1=OP.add)
                        nc.gpsimd.tensor_copy(out=Ppair[:],
                                              in_=ptmp[:].bitcast(f32))
                    pend.append((ps, p, Ppair))
                    mlag = 6 if (ps > 0 and p < 8) else lag
                    if ps == NPASS - 1:
                        mlag = min(mlag, NP - 1 - p)
                    while len(pend) > mlag:
                        emit_pair(pend.popleft())
                while pend:
                    emit_pair(pend.popleft())

            # tail: last pass finish + epilogue
            c0 = 0
            for i, cw in enumerate(tailw):
                fin_chunk(NPASS - 1, c0, cw)
                epi_chunk(NPASS - 1, c0, cw, (c0, cw))
                c0 += cw
            assert c0 == ICW

    nc.compile()
    return nc


def host_inputs(x, gn_w, gn_b, w_qkv, b_qkv, w_out, b_out):
    """Build the 8 per-core input maps from the full problem inputs."""
    import ml_dtypes

    x = np.asarray(x, dtype=np.float32)
    B, _, N = x.shape
    S = N // 2
    w_qkv = np.asarray(w_qkv, np.float32)
    w_out = np.asarray(w_out, np.float32)
    b_qkv = np.asarray(b_qkv, np.float32)
    b_out = np.asarray(b_out, np.float32)
    gn_w = np.asarray(gn_w, np.float32)
    gn_b = np.asarray(gn_b, np.float32)

    # scores = h^T M h + h^T (M^T b + w_k^T b_q); q/k never materialized.
    # wvo composes the out-projection into the v path: v-tilde = wo^T v.
    M = w_qkv[0:C].T @ w_qkv[C:2 * C]
    wvo = (w_out @ w_qkv[2 * C:3 * C]).T
    gidx = np.arange(C) // GS
    gmask = (gidx[:, None] == gidx[None, :]).astype(np.float32) / GS
    bqt = w_qkv[C:2 * C].T @ b_qkv[0:C]
    b_eff = b_out + w_out @ b_qkv[2 * C:3 * C]
    bcat = np.stack([bqt, b_eff, gn_w, gn_b], axis=1)
    wb = np.concatenate([gmask, M, wvo, bcat], axis=1)
    wb = np.ascontiguousarray(wb, np.float32)           # [C, 3C+4]

    in_maps = []
    for core in range(N_CORES):
        b, half = divmod(core, 2)
        xb = np.roll(x[b], -half * S, axis=1)
        in_maps.append({
            "xb": np.ascontiguousarray(xb.astype(ml_dtypes.bfloat16)),
            "xf": np.ascontiguousarray(xb[:, :S]),
            "wb": wb})
    return in_maps


_NC_CACHE = {}
_RUNNER_CACHE = {}


def _make_runner(nc):
    """Compile-once runner: replicates bass2jax.run_bass_via_pjrt but keeps the
    jitted sharded callable so repeat executions skip recompilation."""
    import jax
    import concourse.mybir as mybir
    from jax.sharding import Mesh, PartitionSpec
    from jax.experimental.shard_map import shard_map
    from concourse.bass2jax import (_bass_exec_p, install_neuronx_cc_hook,
                                    partition_id_tensor)

    install_neuronx_cc_hook()
    partition_name = nc.partition_id_tensor.name if nc.partition_id_tensor else None
    in_names, out_names, out_avals, zero_shapes = [], [], [], []
    for alloc in nc.m.functions[0].allocations:
        if not isinstance(alloc, mybir.MemoryLocationSet):
            continue
        name = alloc.memorylocations[0].name
        if alloc.kind == "ExternalInput":
            if name == partition_name:
                continue
            in_names.append(name)
        elif alloc.kind == "ExternalOutput":
            out_names.append(name)
            shape = tuple(alloc.tensor_shape)
            dtype = mybir.dt.np(alloc.dtype)
            out_avals.append(jax.core.ShapedArray(shape, dtype))
            zero_shapes.append((shape, dtype))
    n_params = len(in_names)
    all_names = in_names + out_names
    if partition_name is not None:
        all_names = all_names + [partition_name]
    donate = tuple(range(n_params, n_params + len(out_names)))

    def _body(*args):
        operands = list(args)
        if partition_name is not None:
            operands.append(partition_id_tensor())
        return tuple(_bass_exec_p.bind(
            *operands, out_avals=tuple(out_avals), in_names=tuple(all_names),
            out_names=tuple(out_names), lowering_input_output_aliases=(),
            sim_require_finite=True, sim_require_nnan=True, nc=nc))

    devices = jax.devices()[:N_CORES]
    mesh = Mesh(np.asarray(devices), ("core",))
    specs = (PartitionSpec("core"),)
    sharded = jax.jit(
        shard_map(_body, mesh=mesh,
                  in_specs=specs * (n_params + len(out_names)),
                  out_specs=specs * len(out_names), check_rep=False),
        donate_argnums=donate, keep_unused=True)

    def run(in_maps):
        concat_in = [np.concatenate([np.asarray(m[nm]) for m in in_maps], axis=0)
                     for nm in in_names]
        concat_zeros = [np.zeros((N_CORES * s[0], *s[1:]), d) for s, d in zero_shapes]
        out_arrs = sharded(*concat_in, *concat_zeros)
        out_arrs = [np.asarray(a) for a in out_arrs]
        return [{nm: out_arrs[i].reshape(N_CORES, *out_avals[i].shape)[c]
                 for i, nm in enumerate(out_names)} for c in range(N_CORES)]

    return run


def get_runner(N=4096):
    if N not in _RUNNER_CACHE:
        if N not in _NC_CACHE:
            _NC_CACHE[N] = build(N)
        _RUNNER_CACHE[N] = _make_runner(_NC_CACHE[N])
    return _RUNNER_CACHE[N]


def kernel(x, gn_w, gn_b, w_qkv, b_qkv, w_out, b_out):
    from concourse._compat import axon_active

    x = np.asarray(x, dtype=np.float32)
    B, _, N = x.shape
    S = N // 2
    in_maps = host_inputs(x, gn_w, gn_b, w_qkv, b_qkv, w_out, b_out)
    if axon_active():
        results = get_runner(N)(in_maps)
    else:
        from concourse.bass_utils import run_bass_kernel_spmd

        if N not in _NC_CACHE:
            _NC_CACHE[N] = build(N)
        results = run_bass_kernel_spmd(_NC_CACHE[N], in_maps,
                                       core_ids=list(range(N_CORES))).results
    out = np.empty((B, C, N), dtype=np.float32)
    for core in range(N_CORES):
        b, half = divmod(core, 2)
        out[b, :, half * S:(half + 1) * S] = results[core]["out"]
    return out


# revision 22
# speedup vs baseline: 1.1685x; 1.0028x over previous
"""AttentionBlock (GroupNorm -> QKV -> full attention -> out-proj + residual)
for B=4, C=128, N=4096 on 8 Trainium2 NeuronCores.

Sharding: 8 cores = 4 batches x 2 query-slabs of N/2. Every core runs the
same program; the host rolls each core's x so its query slab is always
columns [0, N/2).

v2 design (vs the f32r baseline):
- QK never materializes q/k: scores = h^T (w_q^T w_k) h + bias, with the
  weight product M composed on the host. Both QK operands are fp8 in a
  packed [64, 2, .] layout so the QK matmuls run in fp8 DoubleRow (0.5
  cyc/col, 2x the f32r rate). h8/qt8 are built in normal [128, .] layout
  and repacked by SBUF->SBUF DMA.
- Softmax row sums come from fp8 DoubleRow all-ones matmuls directly on the
  exp'd P pairs (no DVE pair-add pass at all).
- The exp itself is split across three engines by a per-tile type pattern:
  'A' tiles run true exp on ACT; 'P' tiles compute exp via the Schraudolph
  int32 bit trick on DVE (tensor_scalar mult+add -> i32 = f32 bits) and
  convert to fp8 on GpSimd; 'R' pairs use the 16-bit variant (i16 bits =
  bf16) and feed PV/ones as bf16 moving data (fp8 stationary x bf16 moving
  is legal on PE; fp32 may not mix).
- GroupNorm stats run on a host-shipped bf16 copy of x; the f32 x is only
  loaded (late, overlapped) for the residual. The GN affine is folded into
  the v projection and the QK weights on device (wvAB, wMA).
- finish (1/rowsum) and the out-projection epilogue of pass 0 are deferred
  into pass 1 so only the last chunk's epilogue sits on the tail.
End-to-end relative error vs the fp32 reference ~6e-4 (fp8 + exp-trick).
"""

import math
import sys
from collections import deque

if "/opt/trn_rl_repo" not in sys.path:
    sys.path.insert(0, "/opt/trn_rl_repo")

import numpy as np

C = 128
G = 8
GS = C // G  # channels per group
EPS = 1e-5
N_CORES = 8
SCALE_C = None  # set in build from C


def default_pattern(NP=16, NPASS=4):
    """Per pair-tile (2 j-tiles x 512 queries) exp engine assignment, one
    char per pair-tile per pass. 'F' = f32r fast-path QK + ACT exp (first
    pass warmup); 'A' = ACT exp; 'P' = DVE int-trick + Pool fp8 convert.
    Totals: A 38 (incl 2 F), P 26."""
    p = []
    #        0123456789012345
    p.append("FFFGFGFGFGFGFAFF")  # pass 0: f32r QK; 10F+1A(packed-warm), 5G
    p.append("APAPAPAPAPAPAAPA")  # pass 1: 9A, 7P
    p.append("APAPAPAPAPAPAAPA")  # pass 2: 9A, 7P
    p.append("APAPAPAPAPAAPAAA")  # pass 3: 11A, 5P
    return "".join(p)


def build(N=4096, pattern=None, lag=4, n_junk=0,
          fin_prev_t=(2, 4), tailw=(256, 256)):
    """Build the per-core Bass program. Returns the compiled Bacc module."""
    import concourse.bacc as bacc
    import concourse.bass as bass
    import concourse.mybir as mybir
    import concourse.tile as tile

    f32 = mybir.dt.float32
    f32r = mybir.dt.float32r
    bf16 = mybir.dt.bfloat16
    f8 = mybir.dt.float8e4
    i32 = mybir.dt.int32
    AF = mybir.ActivationFunctionType
    OP = mybir.AluOpType
    DR = mybir.MatmulPerfMode.DoubleRow

    S = N // 2           # query slab width per core
    ICW = 512            # i-chunk width per pass (h2u/rs = 1 PSUM bank each)
    NPASS = S // ICW     # 4 passes over i
    NJT = N // 128
    NP = NJT // 2        # pair-tiles per pass (one [C,1024] slot each)
    BNC = 512            # bn_stats chunk
    NBN = N // BNC
    HCW = 1024           # h8 chunk
    SCALE = 1.0 / math.sqrt(C)
    # Schraudolph exp constants: exp(x) ~ bitcast(int(x*K + B))
    CORR = 0.043677448
    K32 = SCALE * (1 << 23) / math.log(2.0)
    B32 = float((1 << 23) * (127 - CORR))

    if pattern is None:
        pattern = default_pattern(NP, NPASS)
    assert len(pattern) == NPASS * NP

    nc = bacc.Bacc("TRN2", target_bir_lowering=False, debug=False)

    xb_d = nc.dram_tensor("xb", [C, N], bf16, kind="ExternalInput").ap()
    xf_d = nc.dram_tensor("xf", [C, S], f32, kind="ExternalInput").ap()
    # wb = [gmask | M | wvo | bcat(4)]  (wvo = (w_out @ w_v).T, so the
    # out-projection is pre-composed into the v path)
    wb_d = nc.dram_tensor("wb", [C, 3 * C + 4], f32, kind="ExternalInput").ap()
    o_d = nc.dram_tensor("out", [C, S], f32, kind="ExternalOutput").ap()

    with tile.TileContext(nc) as tc:
        with tc.tile_pool(name="consts", bufs=1) as cp, \
             tc.tile_pool(name="big", bufs=1) as bp, \
             tc.tile_pool(name="small", bufs=3) as sp_, \
             tc.tile_pool(name="pP", bufs=9) as pP, \
             tc.tile_pool(name="pT", bufs=5) as pT, \
             tc.tile_pool(name="ps_sT", bufs=3, space="PSUM") as psT, \
             tc.tile_pool(name="ps_rs", bufs=1, space="PSUM") as prs, \
             tc.tile_pool(name="ps_h2", bufs=1, space="PSUM") as ph2:

            # ---- DMA loads (few, fat: HWDGE costs ~625ns per DMA) ----
            xB = bp.tile([C, N], bf16, tag="xB")
            XDC = 1024
            for dc in range(N // XDC):
                nc.sync.dma_start(xB[:, dc * XDC:(dc + 1) * XDC],
                                  xb_d[:, dc * XDC:(dc + 1) * XDC])
            wbS = cp.tile([C, 3 * C + 4], f32, tag="wb")
            nc.sync.dma_start(wbS[:], wb_d[:])
            mS = wbS[:, 0:C]
            wM = wbS[:, C:2 * C]
            wvo = wbS[:, 2 * C:3 * C]
            bS = wbS[:, 3 * C:3 * C + 4]

            # ---- constants ----
            onesF8_2 = cp.tile([C, 2, C], f8, tag="onesf82")
            nc.vector.memset(onesF8_2[:], 1.0)
            epsT = cp.tile([C, 1], f32, tag="eps")
            nc.vector.memset(epsT[:], EPS)
            # dummy act: trigger the exp table load early on ACT
            dumT = cp.tile([C, 1], f32, tag="dum")
            nc.scalar.activation(out=dumT[:], in_=epsT[:], func=AF.Exp, scale=1.0)

            # ---- big SBUF tensors ----
            h8 = bp.tile([C, N], f8, tag="h8")
            hRf = bp.tile([C, N], f32r, tag="hRf")         # f32r pass-0 keys
            qRf = bp.tile([C, ICW], f32r, tag="qRf")       # f32r fast-path qt
            ht2 = bp.tile([64, 2, N], f8, tag="ht2")       # packed QK stationary
            qt8 = bp.tile([C, S], f8, tag="qt8")
            qtt = bp.tile([64, 2, S], f8, tag="qtt")       # packed QK moving
            vTR = bp.tile([C, N], f8, tag="vT")            # [key, chan] (wo-folded)
            xfS = bp.tile([C, S], f32, tag="xf")
            outS = bp.tile([C, S], f32, tag="outS")

            wMA = cp.tile([C, C], bf16, tag="wMA")
            wvAB = cp.tile([C, C], bf16, tag="wvAB")
            aT = sp_.tile([C, 1], f32, tag="aT")
            bT = sp_.tile([C, 1], f32, tag="bT")
            dS = sp_.tile([C, 1], f32, tag="dS")
            beffT = sp_.tile([C, 1], f32, tag="beffT")

            # ================= preamble =================
            # GroupNorm stats on bf16 x
            st6 = sp_.tile([C, NBN, 6], f32, tag="st6")
            for i in range(NBN):
                nc.vector.bn_stats(out=st6[:, i, :],
                                   in_=xB[:, i * BNC:(i + 1) * BNC])
            mv = sp_.tile([C, 2], f32, tag="mv")
            nc.vector.bn_aggr(out=mv[:], in_=st6[:])
            nc.vector.scalar_tensor_tensor(out=mv[:, 1:2], in0=mv[:, 0:1],
                                           scalar=mv[:, 0:1], in1=mv[:, 1:2],
                                           op0=OP.mult, op1=OP.add)
            pre = psT.tile([C, 2 * ICW], f32, tag="sT", name="pre0")
            nc.tensor.matmul(pre[:, 0:2], mS, mv[:], start=True, stop=True)
            gst = sp_.tile([C, 2], f32, tag="gst")
            nc.vector.tensor_copy(gst[:], pre[:, 0:2])
            # xv = eps + gEx2 - gmean^2;  inv = sqrt(1/xv)
            gv = sp_.tile([C, 1], f32, tag="gv")
            nc.vector.scalar_tensor_tensor(out=gv[:], in0=gst[:, 0:1],
                                           scalar=gst[:, 0:1], in1=gst[:, 1:2],
                                           op0=OP.mult, op1=OP.subtract)
            xv = sp_.tile([C, 1], f32, tag="xv")
            nc.vector.tensor_tensor(out=xv[:], in0=epsT[:], in1=gv[:],
                                    op=OP.subtract)
            rxv = sp_.tile([C, 1], f32, tag="rxv")
            nc.vector.reciprocal(out=rxv[:], in_=xv[:])
            inv = sp_.tile([C, 1], f32, tag="inv")
            nc.scalar.activation(out=inv[:], in_=rxv[:], func=AF.Sqrt)
            nc.vector.tensor_tensor(out=aT[:], in0=bS[:, 2:3], in1=inv[:],
                                    op=OP.mult)
            nc.vector.tensor_tensor(out=bT[:], in0=gst[:, 0:1], in1=aT[:],
                                    op=OP.mult)
            nc.vector.tensor_tensor(out=bT[:], in0=bS[:, 3:4], in1=bT[:],
                                    op=OP.subtract)
            # folded weights
            nc.vector.tensor_scalar(out=wMA[:], in0=wM, scalar1=aT[:],
                                    scalar2=None, op0=OP.mult)
            nc.vector.tensor_scalar(out=wvAB[:], in0=wvo, scalar1=aT[:],
                                    scalar2=None, op0=OP.mult)
            # delta = M^T bT + bqt (qt bias); beff += wo@wv@bT = wvo^T bT
            pre2 = psT.tile([C, 2 * ICW], f32, tag="sT", name="pre1")
            nc.tensor.matmul(pre2[:, 0:1], wM, bT[:], start=True, stop=True)
            nc.vector.tensor_tensor(out=dS[:], in0=bS[:, 0:1], in1=pre2[:, 0:1],
                                    op=OP.add)
            nc.tensor.matmul(pre2[:, 4:5], wvo, bT[:], start=True, stop=True)
            nc.vector.tensor_tensor(out=beffT[:], in0=bS[:, 1:2],
                                    in1=pre2[:, 4:5], op=OP.add)

            # fast-path operands: hRf (keys 0:512 f32r), qRf (i 0:512 f32r)
            for rc in range(4):
                nc.vector.tensor_scalar(out=hRf[:, rc * HCW:(rc + 1) * HCW],
                                        in0=xB[:, rc * HCW:(rc + 1) * HCW],
                                        scalar1=aT[:], scalar2=bT[:],
                                        op0=OP.mult, op1=OP.add)

            def qt_mms(cc, dst, col0):
                slx = slice(cc * ICW, (cc + 1) * ICW)
                nc.tensor.matmul(dst[0:64, col0:col0 + ICW], wMA[:, 0:64],
                                 xB[:, slx], start=True, stop=True)
                nc.tensor.matmul(dst[64:128, col0:col0 + ICW], wMA[:, 64:128],
                                 xB[:, slx], start=True, stop=True)

            # qt chunks 0/1 (i-cols 0:1024): qRf (ACT, 512) + fp8 (DVE, 1024)
            qtp01 = psT.tile([C, 2 * ICW], f32, tag="sT", name="qtp01")
            qt_mms(0, qtp01, 0)
            qt_mms(1, qtp01, ICW)
            nc.scalar.activation(out=qRf[:], in_=qtp01[:, 0:ICW],
                                 func=AF.Identity, bias=dS[:], scale=1.0)
            nc.vector.tensor_scalar(out=qt8[:, 0:2 * ICW], in0=qtp01[:],
                                    scalar1=1.0, scalar2=dS[:],
                                    op0=OP.mult, op1=OP.add)
            nc.sync.dma_start(qtt[:, 0, 0:2 * ICW], qt8[0:64, 0:2 * ICW])
            nc.sync.dma_start(qtt[:, 1, 0:2 * ICW], qt8[64:128, 0:2 * ICW])

            # h8 = fp8(aT*xB + bT); repack halves as they complete
            for hc in range(N // HCW):
                sl = slice(hc * HCW, (hc + 1) * HCW)
                nc.vector.tensor_scalar(out=h8[:, sl], in0=xB[:, sl],
                                        scalar1=aT[:], scalar2=bT[:],
                                        op0=OP.mult, op1=OP.add)
                if hc % 2 == 1:
                    sl2 = slice((hc - 1) * HCW, (hc + 1) * HCW)
                    nc.sync.dma_start(ht2[:, 0, sl2], h8[0:64, sl2])
                    nc.sync.dma_start(ht2[:, 1, sl2], h8[64:128, sl2])

            # v-tilde (= wo^T v) chunks: mms into psT slots; chunk 0 on ACT
            def vt_mms(g, vtp):
                for jj in range(HCW // 128):
                    tj = (HCW // 128) * g + jj
                    nc.tensor.matmul(vtp[:, 128 * jj:128 * (jj + 1)],
                                     xB[:, 128 * tj:128 * (tj + 1)], wvAB[:],
                                     start=True, stop=True)

            vtp0 = psT.tile([C, 2 * ICW], f32, tag="sT", name="vtp0")
            vt_mms(0, vtp0)
            nc.scalar.activation(out=vTR[:, 0:HCW], in_=vtp0[:],
                                 func=AF.Identity, scale=1.0)

            # residual x (f32), single fat DMA, overlapped with attention
            nc.sync.dma_start(xfS[:], xf_d[:])
            xpb = bp.tile([C, S], f32, tag="xpb")
            for pc in range(2):
                nc.vector.tensor_scalar(out=xpb[:, pc * S // 2:(pc + 1) * S // 2],
                                        in0=xfS[:, pc * S // 2:(pc + 1) * S // 2],
                                        scalar1=1.0, scalar2=beffT[:],
                                        op0=OP.mult, op1=OP.add)

            # ================= attention =================
            # Each slot holds scores for a PAIR of j-tiles x ICW queries:
            # slot[:, 0:512] = j-tile 2p, slot[:, 512:1024] = j-tile 2p+1.
            # One 1024-wide exp/op1 consumes the slot; PV/rowsum are single
            # 512-wide fp8 DoubleRow matmuls into 1-bank accumulators.
            acc = {}
            pend = deque()

            def emit_pair(job):
                ps, p, ptile = job
                h2p, rsp = acc[ps]
                vpair = vTR[:, 256 * p:256 * (p + 1)].rearrange(
                    "p (two c) -> p two c", two=2)
                nc.tensor.matmul(h2p[:], vpair, ptile[:],
                                 start=p == 0, stop=p == NP - 1, perf_mode=DR)
                nc.tensor.matmul(rsp[:], onesF8_2[:], ptile[:],
                                 start=p == 0, stop=p == NP - 1, perf_mode=DR)

            def fin_chunk(ps, c0, cw):
                # out[:, i] = h2u[:, i]/rs[i] + beff + x
                h2p, rsp = acc[ps]
                sl_i = slice(ps * ICW + c0, ps * ICW + c0 + cw)
                sl_f = slice(c0, c0 + cw)
                recipB = sp_.tile([C, cw], f32, tag=f"recipB{cw}")
                nc.vector.reciprocal_approx_fast(out=recipB[:], in_=rsp[:, sl_f])
                nc.vector.tensor_tensor(out=outS[:, sl_i], in0=h2p[:, sl_f],
                                        in1=recipB[:], op=OP.mult)

            def epi_chunk(ps, c0, cw, dma, eng="pool"):
                sl_i = slice(ps * ICW + c0, ps * ICW + c0 + cw)
                if eng == "pool":
                    nc.gpsimd.tensor_tensor(out=outS[:, sl_i], in0=outS[:, sl_i],
                                            in1=xpb[:, sl_i], op=OP.add)
                else:
                    nc.vector.tensor_tensor(out=outS[:, sl_i], in0=outS[:, sl_i],
                                            in1=xpb[:, sl_i], op=OP.add)
                if dma:
                    d0, dw = dma
                    sl_d = slice(ps * ICW + d0, ps * ICW + d0 + dw)
                    nc.sync.dma_start(o_d[:, sl_d], outS[:, sl_d])

            # in-pass service schedule: (pass, pair) -> list of thunks
            def srv_vt(g, eng):
                def thunk():
                    spt = psT.tile([C, 2 * ICW], f32, tag="sT", name=f"vtp{g}")
                    vt_mms(g, spt)
                    if eng == "act":
                        nc.scalar.activation(out=vTR[:, HCW * g:HCW * (g + 1)],
                                             in_=spt[:], func=AF.Identity,
                                             scale=1.0)
                    else:
                        nc.vector.tensor_copy(vTR[:, HCW * g:HCW * (g + 1)],
                                              spt[:])
                return thunk

            def srv_qt_hi():
                def thunk():
                    spt = psT.tile([C, 2 * ICW], f32, tag="sT", name="qtp23")
                    qt_mms(2, spt, 0)
                    qt_mms(3, spt, ICW)
                    nc.scalar.activation(out=qt8[:, 2 * ICW:S], in_=spt[:],
                                         func=AF.Identity, bias=dS[:],
                                         scale=1.0)
                    nc.sync.dma_start(qtt[:, 0, 2 * ICW:S], qt8[0:64, 2 * ICW:S])
                    nc.sync.dma_start(qtt[:, 1, 2 * ICW:S],
                                      qt8[64:128, 2 * ICW:S])
                return thunk

            srv = {(0, 4): [srv_vt(1, "act")], (0, 7): [srv_vt(2, "act")],
                   (0, 10): [srv_vt(3, "act")], (0, 13): [srv_qt_hi()]}
            for ps in range(1, NPASS):
                for i, q in enumerate(fin_prev_t):
                    srv.setdefault((ps, q), []).append(
                        lambda ps=ps, i=i: (fin_chunk(ps - 1, i * 256, 256),
                                            epi_chunk(ps - 1, i * 256, 256,
                                                      (0, ICW) if i == 1
                                                      else None)))

            for ps in range(NPASS):
                acc[ps] = (ph2.tile([C, ICW], f32, tag="h2u", name=f"h2u{ps}"),
                           prs.tile([C, ICW], f32, tag="rs", name=f"rs{ps}"))
                ic0 = ps * ICW
                for p in range(NP):
                    ty = pattern[ps * NP + p]
                    for thunk in srv.get((ps, p), ()):
                        thunk()
                    # QK for j-tiles 2p, 2p+1 into one [C, 1024] slot
                    sT = psT.tile([C, 2 * ICW], f32, tag="sT")
                    for k in range(2):
                        t = 2 * p + k
                        ks = slice(k * ICW, (k + 1) * ICW)
                        if ty in "FG":
                            nc.tensor.matmul(
                                sT[:, ks], hRf[:, 128 * t:128 * (t + 1)],
                                qRf[:], start=True, stop=True)
                        else:
                            nc.tensor.matmul(
                                sT[:, ks], ht2[:, :, 128 * t:128 * (t + 1)],
                                qtt[:, :, ic0:ic0 + ICW],
                                start=True, stop=True, perf_mode=DR)
                    # exp: one 1024-wide op -> Ppair [C, 2, 512] fp8
                    Ppair = pP.tile([C, 2, ICW], f8, tag="P", name=f"P{ps}_{p}")
                    if ty in "AF":
                        nc.scalar.activation(out=Ppair[:], in_=sT[:],
                                             func=AF.Exp, scale=SCALE)
                    elif ty == "G":
                        ptmp = pT.tile([C, 2 * ICW], i32, tag="Ptmp")
                        nc.vector.tensor_scalar(out=ptmp[:], in0=sT[:],
                                                scalar1=K32, scalar2=B32,
                                                op0=OP.mult, op1=OP.add)
                        nc.gpsimd.tensor_copy(out=Ppair[:],
                                              in_=ptmp[:].bitcast(f32))
                    else:  # P
                        ptmp = pT.tile([C, 2 * ICW], i32, tag="Ptmp")
                        nc.vector.tensor_scalar(out=ptmp[:], in0=sT[:],
                                                scalar1=K32, scalar2=B32,
                                                op0=OP.mult, op1=OP.add)
                        nc.gpsimd.tensor_copy(out=Ppair[:],
                                              in_=ptmp[:].bitcast(f32))
                    pend.append((ps, p, Ppair))
                    mlag = 6 if (ps > 0 and p < 8) else lag
                    mlag = min(mlag, NP - 1 - p)
                    while len(pend) > mlag:
                        emit_pair(pend.popleft())
                while pend:
                    emit_pair(pend.popleft())

            # tail: last pass finish + epilogue
            c0 = 0
            for i, cw in enumerate(tailw):
                fin_chunk(NPASS - 1, c0, cw)
                epi_chunk(NPASS - 1, c0, cw, (c0, cw), eng="dve")
                c0 += cw
            assert c0 == ICW

    nc.compile()
    return nc


def host_inputs(x, gn_w, gn_b, w_qkv, b_qkv, w_out, b_out):
    """Build the 8 per-core input maps from the full problem inputs."""
    import ml_dtypes

    x = np.asarray(x, dtype=np.float32)
    B, _, N = x.shape
    S = N // 2
    w_qkv = np.asarray(w_qkv, np.float32)
    w_out = np.asarray(w_out, np.float32)
    b_qkv = np.asarray(b_qkv, np.float32)
    b_out = np.asarray(b_out, np.float32)
    gn_w = np.asarray(gn_w, np.float32)
    gn_b = np.asarray(gn_b, np.float32)

    # scores = h^T M h + h^T (M^T b + w_k^T b_q); q/k never materialized.
    # wvo composes the out-projection into the v path: v-tilde = wo^T v.
    M = w_qkv[0:C].T @ w_qkv[C:2 * C]
    wvo = (w_out @ w_qkv[2 * C:3 * C]).T
    gidx = np.arange(C) // GS
    gmask = (gidx[:, None] == gidx[None, :]).astype(np.float32) / GS
    bqt = w_qkv[C:2 * C].T @ b_qkv[0:C]
    b_eff = b_out + w_out @ b_qkv[2 * C:3 * C]
    bcat = np.stack([bqt, b_eff, gn_w, gn_b], axis=1)
    wb = np.concatenate([gmask, M, wvo, bcat], axis=1)
    wb = np.ascontiguousarray(wb, np.float32)           # [C, 3C+4]

    in_maps = []
    for core in range(N_CORES):
        b, half = divmod(core, 2)
        xb = np.roll(x[b], -half * S, axis=1)
        in_maps.append({
            "xb": np.ascontiguousarray(xb.astype(ml_dtypes.bfloat16)),
            "xf": np.ascontiguousarray(xb[:, :S]),
            "wb": wb})
    return in_maps


_NC_CACHE = {}
_RUNNER_CACHE = {}


def _make_runner(nc):
    """Compile-once runner: replicates bass2jax.run_bass_via_pjrt but keeps the
    jitted sharded callable so repeat executions skip recompilation."""
    import jax
    import concourse.mybir as mybir
    from jax.sharding import Mesh, PartitionSpec
    from jax.experimental.shard_map import shard_map
    from concourse.bass2jax import (_bass_exec_p, install_neuronx_cc_hook,
                                    partition_id_tensor)

    install_neuronx_cc_hook()
    partition_name = nc.partition_id_tensor.name if nc.partition_id_tensor else None
    in_names, out_names, out_avals, zero_shapes = [], [], [], []
    for alloc in nc.m.functions[0].allocations:
        if not isinstance(alloc, mybir.MemoryLocationSet):
            continue
        name = alloc.memorylocations[0].name
        if alloc.kind == "ExternalInput":
            if name == partition_name:
                continue
            in_names.append(name)
        elif alloc.kind == "ExternalOutput":
            out_names.append(name)
            shape = tuple(alloc.tensor_shape)
            dtype = mybir.dt.np(alloc.dtype)
            out_avals.append(jax.core.ShapedArray(shape, dtype))
            zero_shapes.append((shape, dtype))
    n_params = len(in_names)
    all_names = in_names + out_names
    if partition_name is not None:
        all_names = all_names + [partition_name]
    donate = tuple(range(n_params, n_params + len(out_names)))

    def _body(*args):
        operands = list(args)
        if partition_name is not None:
            operands.append(partition_id_tensor())
        return tuple(_bass_exec_p.bind(
            *operands, out_avals=tuple(out_avals), in_names=tuple(all_names),
            out_names=tuple(out_names), lowering_input_output_aliases=(),
            sim_require_finite=True, sim_require_nnan=True, nc=nc))

    devices = jax.devices()[:N_CORES]
    mesh = Mesh(np.asarray(devices), ("core",))
    specs = (PartitionSpec("core"),)
    sharded = jax.jit(
        shard_map(_body, mesh=mesh,
                  in_specs=specs * (n_params + len(out_names)),
                  out_specs=specs * len(out_names), check_rep=False),
        donate_argnums=donate, keep_unused=True)

    def run(in_maps):
        concat_in = [np.concatenate([np.asarray(m[nm]) for m in in_maps], axis=0)
                     for nm in in_names]
        concat_zeros = [np.zeros((N_CORES * s[0], *s[1:]), d) for s, d in zero_shapes]
        out_arrs = sharded(*concat_in, *concat_zeros)
        out_arrs = [np.asarray(a) for a in out_arrs]
        return [{nm: out_arrs[i].reshape(N_CORES, *out_avals[i].shape)[c]
                 for i, nm in enumerate(out_names)} for c in range(N_CORES)]

    return run


def get_runner(N=4096):
    if N not in _RUNNER_CACHE:
        if N not in _NC_CACHE:
            _NC_CACHE[N] = build(N)
        _RUNNER_CACHE[N] = _make_runner(_NC_CACHE[N])
    return _RUNNER_CACHE[N]


def kernel(x, gn_w, gn_b, w_qkv, b_qkv, w_out, b_out):
    from concourse._compat import axon_active

    x = np.asarray(x, dtype=np.float32)
    B, _, N = x.shape
    S = N // 2
    in_maps = host_inputs(x, gn_w, gn_b, w_qkv, b_qkv, w_out, b_out)
    if axon_active():
        results = get_runner(N)(in_maps)
    else:
        from concourse.bass_utils import run_bass_kernel_spmd

        if N not in _NC_CACHE:
            _NC_CACHE[N] = build(N)
        results = run_bass_kernel_spmd(_NC_CACHE[N], in_maps,
                                       core_ids=list(range(N_CORES))).results
    out = np.empty((B, C, N), dtype=np.float32)
    for core in range(N_CORES):
        b, half = divmod(core, 2)
        out[b, :, half * S:(half + 1) * S] = results[core]["out"]
    return out
